# revision 1
# baseline (speedup 1.0000x reference)
"""Trainium2 Bass kernel for nn_AttentionModule (dense_transformer).

Reference computation (per batch sample b):
    theta = sigmoid(x @ Wt + bt)            # [N, F]
    phi   = x @ Wp + bp                     # [N, F]
    att   = theta @ phi.T                   # [N(n), N(m)]
    att   = softmax(att, axis over n)       # softmax over QUERY axis
    out   = att(n,m) @ x(m,d) + x           # [N, D]
  (the g = tanh(x@Wg+bg) branch is dead — never used in the output)

Strategy: pure data parallelism — B=8 samples, one per NeuronCore. No
collectives. Per core, everything is computed in transposed score layout
ST[m, n] = phi[m]·theta[n], so the softmax axis (n) is the free axis.
Softmax runs WITHOUT max-subtraction: logits for this problem's input
distribution peak at ~57 and a constant -20 shift (cancelled by the
normalization) puts fp32 exp overflow at logit 108.7, so exp(ST-20) is
safe; this removes the reduce_max chain from the critical path. The
normalization is applied by scaling E rows in place (per-partition
scalar on DVE): A[m, n] = E[m, n] / s[m].

All matmuls run in bf16 (fp32 PSUM accumulation): validated rel_l2 err
~7e-3 vs fp32 reference, and bf16 is 4x the fp32 TensorE throughput.

Scheduling notes (walrus sync-wait limits + Tile dep granularity):
 - built as bacc.Bacc: finalize() runs generate_event_semaphores, which
   legalizes multi-sem waits (TPB instructions carry at most one);
 - every SBUF tile is written by exactly ONE dma_start, and tiles are
   split to match consumer granularity (deps are tile-granular);
 - SBUF pools never overlap/reuse address space (a tile allocated over a
   freed region inherits WAR waits against all old accessor procs).
"""

import numpy as np
import ml_dtypes

import concourse.bass as bass
import concourse.bacc as bacc
import concourse.mybir as mybir
from concourse.tile import TileContext
from concourse.bass_utils import run_bass_kernel_spmd

P = 128
B, N, D, F = 8, 2048, 1024, 512
NCH = N // P   # 16 chunks of the token dim
DCH = D // P   # 8 chunks of the model dim
FCH = F // P   # 4 chunks of the filter dim
NF = 512       # matmul moving free dim (one fp32 PSUM bank)
NSL = N // NF  # 4 score column slices
DSL = D // NF  # 2 output d slices

BF16 = mybir.dt.bfloat16
F32 = mybir.dt.float32
AX = mybir.AxisListType.X
AF = mybir.ActivationFunctionType


def build_bass():
    nc = bacc.Bacc()

    xT_d = nc.declare_dram_parameter("xT", [D, N], BF16, isOutput=False)
    xn_d = nc.declare_dram_parameter("xn", [N, D], BF16, isOutput=False)
    xr_d = nc.declare_dram_parameter("xr", [N, D], F32, isOutput=False)
    # weights pre-swizzled on host. Wt is the exact concatenated SBUF tile
    # image [Wt0a | Wt0b | Wt1 | Wt2 | Wt3] so every DMA descriptor run is
    # >=1KB contiguous (sub-512B runs pay a 2x read-modify-write penalty) and
    # each fc block lands as early as its first consumer group needs it.
    Wt_d = nc.declare_dram_parameter("Wt", [P, DCH * F], BF16, isOutput=False)
    Wp_d = nc.declare_dram_parameter("Wp", [P, DCH, F], BF16, isOutput=False)
    bt_d = nc.declare_dram_parameter("bt", [P, FCH], F32, isOutput=False)
    bp_d = nc.declare_dram_parameter("bp", [P, FCH], F32, isOutput=False)
    out_d = nc.declare_dram_parameter("out", [N, D], F32, isOutput=True)

    with TileContext(nc) as tc:
        with (
            tc.tile_pool(name="const", bufs=1) as cpool,
            tc.tile_pool(name="mid", bufs=1) as mid,
            tc.tile_pool(name="big", bufs=1) as bigp,
            tc.tile_pool(name="stats", bufs=16) as stats,
            tc.tile_pool(name="xst", bufs=3) as xstp,
            tc.tile_pool(name="ost", bufs=3) as ostp,
            tc.tile_pool(name="psum", bufs=8, space="PSUM") as psum,
        ):
            # coalesced input tiles: ONE large DMA each (the HWDGE pipe is
            # serial with a ~0.6us per-DMA floor, so fewer/bigger wins),
            # sized to match consumption granularity (per ns-slice for xT)
            # startup-critical tiles split in dc-halves: the first 4 matmuls
            # need only Wt[fc0, dc0-3] + xT[ns0, dc0-3] (~640KB)
            HD = DCH // 2
            Wt0_s = [cpool.tile([P, HD * P], BF16, name=f"wt0{h}", tag=f"wt0{h}")
                     for h in range(2)]
            WtK_s = [cpool.tile([P, DCH * P], BF16, name=f"wtk{k}",
                                tag=f"wtk{k}") for k in range(1, FCH)]
            Wp_s = cpool.tile([P, DCH, F], BF16, name="wps", tag="wps")
            bt_s = cpool.tile([P, FCH], F32, name="bts", tag="bts")
            bp_s = cpool.tile([P, FCH], F32, name="bps", tag="bps")
            xT0_s = [cpool.tile([P, HD, NF], BF16, name=f"xt0{h}",
                                tag=f"xt0{h}") for h in range(2)]
            xT_s = [None] + [cpool.tile([P, DCH, NF], BF16, name=f"xts{ns}",
                                        tag=f"xts{ns}") for ns in range(1, NSL)]

            def xt_dc(ns, dc):
                if ns == 0:
                    return xT0_s[dc // HD][:, dc % HD]
                return xT_s[ns][:, dc]
            XNG = min(8, NCH)  # xn m-chunks per tile (swept: 8 optimal)
            xn_s = [cpool.tile([P, XNG, D], BF16, name=f"xns{g}",
                               tag=f"xns{g}") for g in range(NCH // XNG)]
            th_s = mid.tile([P, FCH, N], BF16, name="ths")  # thetaT: [f, n]
            ph_s = mid.tile([P, FCH, N], BF16, name="phs")  # phiT:   [f, m]
            # E (scaled to A in place), one tile per m-chunk
            e_s = [bigp.tile([P, N], BF16, name=f"es{mc}", tag=f"es{mc}")
                   for mc in range(NCH)]

            Wp_r = Wp_d[:]
            xT_r = xT_d[:].rearrange("(c p) n -> p c n", p=P)
            xn_r = xn_d[:].rearrange("(c p) d -> p c d", p=P)
            # the cost model treats HWDGE as one serial FIFO pipe with a
            # ~0.6us floor per dma_start: use FEW, LARGE DMAs, strictly in
            # first-use order (xn last: not needed until phase 3)
            # PE warm-up: the HAM clock gate holds PE at 1.2GHz until ~3.4us
            # of sustained activity. The first real matmul waits ~3.6us for
            # DMA anyway, so burn that idle time on dummy matmuls over memset
            # tiles — the real stream then starts at 2.4GHz. (No cost in the
            # timeline model: PE was idle.)
            zx = cpool.tile([P, NF], BF16, name="zx", tag="zx")
            nc.vector.memset(zx, 0)
            eb_s = cpool.tile([P, 1], F32, name="ebs", tag="ebs")
            nc.vector.memset(eb_s, -20.0)
            zp = psum.tile([P, NF], F32, name="pst", tag="pst")
            for i in range(8):
                nc.tensor.matmul(zp, zx[:, 0:P], zx, start=(i == 0),
                                 stop=(i == 7))

            HP = HD * P  # 512: one Wt0 half-image width
            DP = DCH * P  # 1024: one WtK fc-block image width
            nc.sync.dma_start(out=Wt0_s[0], in_=Wt_d[:, 0:HP])
            nc.sync.dma_start(out=xT0_s[0], in_=xT_r[:, 0:HD, 0:NF])
            nc.sync.dma_start(out=Wt0_s[1], in_=Wt_d[:, HP:2 * HP])
            for k in range(1, FCH):  # all remaining fc blocks before xt0b
                nc.sync.dma_start(out=WtK_s[k - 1],
                                  in_=Wt_d[:, k * DP:(k + 1) * DP])
            nc.sync.dma_start(out=xT0_s[1], in_=xT_r[:, HD:DCH, 0:NF])
            nc.sync.dma_start(out=xT_s[1], in_=xT_r[:, :, NF:2 * NF])
            # biases ride after xT1: the first sigmoid can lag (8 psum slots
            # of runway) but the ns=1 matmul group cannot
            nc.sync.dma_start(out=bt_s, in_=bt_d[:])
            nc.sync.dma_start(out=bp_s, in_=bp_d[:])
            for ns in range(2, NSL):
                nc.sync.dma_start(out=xT_s[ns],
                                  in_=xT_r[:, :, ns * NF:(ns + 1) * NF])
            nc.sync.dma_start(out=Wp_s, in_=Wp_r)
            for g in range(NCH // XNG):
                nc.sync.dma_start(out=xn_s[g],
                                  in_=xn_r[:, g * XNG:(g + 1) * XNG])

            # ---------------- Phase 1: projections ----------------
            # thT[f, n] = sigmoid(sum_d Wt[d, f] * xT[d, n] + bt[f])
            # phT[f, n] =         sum_d Wp[d, f] * xT[d, n] + bp[f]
            for ns in range(NSL):
                nsl = slice(ns * NF, (ns + 1) * NF)
                for fc in range(FCH):
                    ps = psum.tile([P, NF], F32, name="pst", tag="pst")
                    for dc in range(DCH):
                        if fc == 0:
                            w0 = (dc % HD) * P
                            wt_fc = Wt0_s[dc // HD][:, w0:w0 + P]
                        else:
                            w0 = dc * P
                            wt_fc = WtK_s[fc - 1][:, w0:w0 + P]
                        nc.tensor.matmul(
                            ps,
                            wt_fc,
                            xt_dc(ns, dc),
                            start=(dc == 0),
                            stop=(dc == DCH - 1),
                        )
                    nc.scalar.activation(
                        th_s[:, fc, nsl], ps, AF.Sigmoid,
                        bias=bt_s[:, fc:fc + 1],
                    )
            for ns in range(NSL):
                nsl = slice(ns * NF, (ns + 1) * NF)
                for fc in range(FCH):
                    ps = psum.tile([P, NF], F32, name="pst", tag="pst")
                    for dc in range(DCH):
                        nc.tensor.matmul(
                            ps,
                            Wp_s[:, dc, fc * P:(fc + 1) * P],
                            xt_dc(ns, dc),
                            start=(dc == 0),
                            stop=(dc == DCH - 1),
                        )
                    nc.vector.tensor_scalar_add(
                        ph_s[:, fc, nsl], ps, bp_s[:, fc:fc + 1]
                    )

            # ------------- Phase 2: scores + row softmax -------------
            # ST[m, n] = sum_f phT[f, m] * thT[f, n]  (one m-block at a time,
            # in 512-wide slices: exp+accum per slice, partials summed on DVE)
            # A[m, n] = exp(ST) / sum_n exp(ST)   (no max-sub: logits < ~60)
            for mc in range(NCH):
                for ns in range(NSL):
                    nsl = slice(ns * NF, (ns + 1) * NF)
                    st = psum.tile([P, NF], F32, name="pst", tag="pst")
                    for fc in range(FCH):
                        nc.tensor.matmul(
                            st,
                            ph_s[:, fc, mc * P:(mc + 1) * P],
                            th_s[:, fc, nsl],
                            start=(fc == 0),
                            stop=(fc == FCH - 1),
                        )
                    # constant shift: normalization cancels it; moves fp32
                    # exp overflow from logit 88.7 to 108.7
                    nc.scalar.activation(
                        e_s[mc][:, nsl], st, AF.Exp, bias=eb_s,
                    )
                rowsum = stats.tile([P, 1], F32, name="rowsum", tag="rowsum")
                nc.vector.reduce_sum(rowsum, e_s[mc], axis=AX)
                recip = stats.tile([P, 1], F32, name="recip", tag="recip")
                nc.vector.reciprocal(recip, rowsum)
                nc.vector.tensor_scalar_mul(e_s[mc], e_s[mc], recip)

            # ------------- Phase 3: weighted sum + residual -------------
            # out[n, d] = sum_m A[m, n] * xn[m, d] + x[n, d]
            for nch in range(NCH):
                xr_t = xstp.tile([P, D], F32, name="xrt", tag="xrt")
                nc.sync.dma_start(
                    out=xr_t, in_=xr_d[nch * P:(nch + 1) * P, :],
                )
                for dsl in range(DSL):
                    last = (nch == NCH - 1 and dsl == DSL - 1)
                    # the very last tile runs as a 384+128 pair: the wide
                    # piece's add+store overlaps the narrow piece's matmuls,
                    # and the final store's scalable costs shrink to N=128
                    pieces = [NF - P, P] if last else [NF]
                    d0 = dsl * NF
                    for hw_ in pieces:
                        dslc = slice(d0, d0 + hw_)
                        d0 += hw_
                        o_ps = psum.tile([P, hw_], F32, name="pst", tag="pst")
                        for mc in range(NCH):
                            nc.tensor.matmul(
                                o_ps,
                                e_s[mc][:, nch * P:(nch + 1) * P],
                                xn_s[mc // XNG][:, mc % XNG, dslc],
                                start=(mc == 0),
                                stop=(mc == NCH - 1),
                            )
                        o_sb = ostp.tile([P, hw_], F32, name="osb", tag="osb")
                        nc.vector.tensor_add(o_sb, o_ps, xr_t[:, dslc])
                        nc.sync.dma_start(
                            out=out_d[nch * P:(nch + 1) * P, dslc],
                            in_=o_sb,
                        )
    nc.finalize()  # Bacc legalization passes (wait splitting, reg alloc, ...)
    return nc


_NC = None


def _get_nc():
    global _NC
    if _NC is None:
        _NC = build_bass()
    return _NC


def make_in_maps(x, Wt, bt, Wp, bp):
    bf16 = ml_dtypes.bfloat16

    def swz(W):
        # [D, F] -> [P, DCH, F] so SBUF partition p reads one contiguous run
        dch = W.shape[0] // P
        w = np.asarray(W, np.float32).astype(bf16)
        return np.ascontiguousarray(w.reshape(dch, P, -1).transpose(1, 0, 2))

    def wt_image(W):
        # exact SBUF image [Wt0a | Wt0b | Wt1 | Wt2 | ...] per partition row:
        #   h-half of fc0:   [P, HD*P]  from W[h*HD:(h+1)*HD, :, 0:P]
        #   each fc>=1 block: [P, DCH*P]  dc-major
        dch = W.shape[0] // P
        hd = dch // 2
        w = np.asarray(W, np.float32).astype(bf16).reshape(dch, P, -1)
        fch = w.shape[2] // P
        parts = [
            w[h * hd:(h + 1) * hd, :, 0:P].transpose(1, 0, 2).reshape(P, hd * P)
            for h in range(2)
        ]
        for k in range(1, fch):
            parts.append(
                w[:, :, k * P:(k + 1) * P].transpose(1, 0, 2).reshape(P, dch * P))
        return np.ascontiguousarray(np.concatenate(parts, axis=1))

    Wt16 = wt_image(Wt)
    Wp16 = swz(Wp)
    # bias layout [P, FCH]: bt_r[p, c] = bt[c*P + p]
    fch = bt.size // P
    bt_r = np.ascontiguousarray(np.asarray(bt, np.float32).reshape(fch, P).T)
    bp_r = np.ascontiguousarray(np.asarray(bp, np.float32).reshape(fch, P).T)
    in_maps = []
    for b in range(x.shape[0]):
        xb = np.ascontiguousarray(np.asarray(x[b], np.float32))
        xb16 = xb.astype(bf16)
        in_maps.append({
            "xT": np.ascontiguousarray(xb16.T),
            "xn": xb16,
            "xr": xb,
            "Wt": Wt16,
            "Wp": Wp16,
            "bt": bt_r,
            "bp": bp_r,
        })
    return in_maps


def run(inputs, trace=False):
    """Run on 8 NeuronCores; returns (out [B,N,D] f32, BassKernelResults)."""
    x = inputs["x"]
    assert x.shape == (B, N, D), x.shape
    nc = _get_nc()
    in_maps = make_in_maps(x, inputs["Wt"], inputs["bt"], inputs["Wp"], inputs["bp"])
    res = run_bass_kernel_spmd(nc, in_maps, core_ids=list(range(B)), trace=trace)
    out = np.stack([res.results[c]["out"] for c in range(B)], axis=0)
    return out.astype(np.float32), res


def kernel(**inputs) -> np.ndarray:
    out, _ = run(inputs)
    return out



# revision 4
# speedup vs baseline: 1.2489x; 1.2489x over previous
"""Trainium2 Bass kernel for nn_AttentionModule (dense_transformer), fp8 DR.

Reference computation (per batch sample b):
    theta = sigmoid(x @ Wt + bt)            # [N, F]
    phi   = x @ Wp + bp                     # [N, F]
    att   = theta @ phi.T                   # [N(n), N(m)]
    att   = softmax(att, axis over n)       # softmax over QUERY axis
    out   = att(n,m) @ x(m,d) + x           # [N, D]

Strategy: pure data parallelism (B=8 samples, one per core, no
collectives) + fp8-e4m3 DoubleRow matmuls (0.5 cycles/row, 2x128
contraction per instruction = 4x bf16 MAC rate in the cost model).

Precision scheme (validated in numerics2.py, rel-l2 vs f64 ref):
 - every bf16 matmul X@W is replaced by 2-3 fp8 terms
       X8@W8 + X8@Wr + Xr@W8      (r = unscaled e4m3 residual)
   accumulated in the same fp32 PSUM group; residual pairs restore
   ~bf16-equivalent precision at 0.5-0.75x of bf16 matmul time.
 - weights are pre-scaled by 32 on host (W' = 32W) so their e4m3
   residuals land in normal range; the 1/32 is folded into the
   activation scale.
 - theta is carried as T = tanh(z/2) = 2*sigmoid(z)-1 in (-1,1):
   the scores logit becomes 0.5*(T.phi) + 0.5*sum_f(phi[m]); the
   second term is constant per m and cancels in the softmax over n
   (exp bias -20; measured exponent range on real inputs [-8, 11.1]).
 - att is quantized to e4m3 AFTER normalization; weighted uses
   2 terms (att8@x8 + att8@xr8, rel~1.38e-2) or 3 terms (+Ar@x8,
   rel~5.5e-3) per WEIGHTED_TERMS.

Scheduling:
 - PE work (cost model, full clock): proj 41us + scores 41us +
   weighted 55us (2t) = 137us vs 218us bf16 roofline.
 - exp runs on ACT with accum_out giving row-sums for free; att8
   normalization on ACT (Copy with per-partition scale=recip).
 - tiles are split to consumer granularity (deps are tile-granular):
   T/P operands per (fc-pair, ns-512), att8/Ar per m-chunk-pair.
 - SBUF: phase-1 operand pool (48KB) is closed after phase 1 and its
   space reused for the phase-3 x tiles (LIFO pool stack); E lives in
   a rotating 4-deep pool (each E[mc] dies once att8[mc] is built).
"""

import numpy as np
import ml_dtypes

import concourse.bass as bass
import concourse.bacc as bacc
import concourse.mybir as mybir
from concourse.tile import TileContext
from concourse.bass_utils import run_bass_kernel_spmd

P = 128
B, N, D, F = 8, 2048, 1024, 512
NCH = N // P    # 16 m/n chunks
DCH = D // P    # 8 d chunks
FCH = F // P    # 4 f chunks
NF = 512        # psum moving width (one fp32 bank)
NSL = N // NF   # 4 column slices
DSL = D // NF   # 2 output d slices
WSCALE = 32.0   # host weight pre-scale (residuals out of denormals)

WEIGHTED_TERMS = 2   # 2: att8@(x8+xr8) ~1.4e-2 | 3: +Ar@x8 ~5.5e-3

BF16 = mybir.dt.bfloat16
F32 = mybir.dt.float32
E4 = mybir.dt.float8e4
AX = mybir.AxisListType.X
AF = mybir.ActivationFunctionType
DR = mybir.MatmulPerfMode.DoubleRow
ALU = mybir.AluOpType
E4NP = ml_dtypes.float8_e4m3


def build_bass():
    nc = bacc.Bacc()

    wt8_d = nc.declare_dram_parameter("wt8", [P, DCH, F], E4, isOutput=False)
    wtr_d = nc.declare_dram_parameter("wtr8", [P, DCH, F], E4, isOutput=False)
    wp8_d = nc.declare_dram_parameter("wp8", [P, DCH, F], E4, isOutput=False)
    wpr_d = nc.declare_dram_parameter("wpr8", [P, DCH, F], E4, isOutput=False)
    bt2_d = nc.declare_dram_parameter("bt2", [P, FCH], F32, isOutput=False)
    bp_d = nc.declare_dram_parameter("bp", [P, FCH], F32, isOutput=False)
    xt8_d = [nc.declare_dram_parameter(f"xt8{ns}", [P, DCH, NF], E4,
                                       isOutput=False) for ns in range(NSL)]
    xtr_d = [nc.declare_dram_parameter(f"xtr8{ns}", [P, DCH, NF], E4,
                                       isOutput=False) for ns in range(NSL)]
    xn8_d = nc.declare_dram_parameter("xn8", [P, NCH, D], E4, isOutput=False)
    xnr_d = nc.declare_dram_parameter("xnr8", [P, NCH, D], E4, isOutput=False)
    xr_d = nc.declare_dram_parameter("xr", [N, D], BF16, isOutput=False)
    out_d = nc.declare_dram_parameter("out", [N, D], F32, isOutput=True)

    with TileContext(nc) as tc:
        from contextlib import ExitStack
        es = ExitStack()
        cpool = es.enter_context(tc.tile_pool(name="const", bufs=1))
        stats = es.enter_context(tc.tile_pool(name="stats", bufs=4))
        epool = es.enter_context(tc.tile_pool(name="ep", bufs=4))
        apool = es.enter_context(tc.tile_pool(name="a8", bufs=1))
        appool = es.enter_context(tc.tile_pool(name="apre", bufs=2))
        xrp = es.enter_context(tc.tile_pool(name="xrp", bufs=3))
        outp = es.enter_context(tc.tile_pool(name="outp", bufs=3))
        tpp = es.enter_context(tc.tile_pool(name="tpp", bufs=1))
        tst = es.enter_context(tc.tile_pool(name="tst", bufs=3))
        psum = es.enter_context(tc.tile_pool(name="psum", bufs=8, space="PSUM"))
        ph1cm = tc.tile_pool(name="ph1", bufs=1)
        ph1 = ph1cm.__enter__()

        # ---- constants / stats ----
        bt2_s = cpool.tile([P, FCH], F32, name="bt2s", tag="bt2s")
        bp_s = cpool.tile([P, FCH], F32, name="bps", tag="bps")
        zx = cpool.tile([P, NF], BF16, name="zx", tag="zx")
        nc.vector.memset(zx, 0)
        eb_s = cpool.tile([P, 1], F32, name="ebs", tag="ebs")
        nc.vector.memset(eb_s, -20.0)

        # PE warm-up: the pstate ramp holds PE below 2.4GHz for ~3.4us of
        # sustained activity; the first real matmul waits on DMA anyway, so
        # burn the idle time on dummy matmuls (costless: PE was idle).
        zp = psum.tile([P, NF], F32, name="pst", tag="pst")
        for i in range(8):
            nc.tensor.matmul(zp, zx[:, 0:P], zx, start=(i == 0), stop=(i == 7))

        # ---- phase-1 operand tiles + DMAs (first-use order) ----
        wt8_s = ph1.tile([P, DCH, F], E4, name="wt8s", tag="wt8s")
        wtr_s = ph1.tile([P, DCH, F], E4, name="wtrs", tag="wtrs")
        wp8_s = ph1.tile([P, DCH, F], E4, name="wp8s", tag="wp8s")
        wpr_s = ph1.tile([P, DCH, F], E4, name="wprs", tag="wprs")
        xt8_s = [ph1.tile([P, DCH, NF], E4, name=f"xt8{ns}", tag=f"xt8{ns}")
                 for ns in range(NSL)]
        xtr_s = [ph1.tile([P, DCH, NF], E4, name=f"xtr{ns}", tag=f"xtr{ns}")
                 for ns in range(NSL)]

        nc.sync.dma_start(out=wt8_s, in_=wt8_d[:])
        nc.sync.dma_start(out=xt8_s[0], in_=xt8_d[0][:])
        nc.sync.dma_start(out=bt2_s, in_=bt2_d[:])
        nc.sync.dma_start(out=wtr_s, in_=wtr_d[:])
        nc.sync.dma_start(out=xtr_s[0], in_=xtr_d[0][:])
        nc.sync.dma_start(out=wp8_s, in_=wp8_d[:])
        nc.sync.dma_start(out=wpr_s, in_=wpr_d[:])
        nc.sync.dma_start(out=bp_s, in_=bp_d[:])
        for ns in range(1, NSL):
            nc.sync.dma_start(out=xt8_s[ns], in_=xt8_d[ns][:])
            nc.sync.dma_start(out=xtr_s[ns], in_=xtr_d[ns][:])

        # T/P operand tiles: per (fc-pair, ns-512) so scores(mc, ns) only
        # waits on the exact phase-1 blocks it reads.
        FCP = FCH // 2
        t8_s = [[tpp.tile([P, 2, NF], E4, name=f"t8_{j}_{ns}",
                          tag=f"t8_{j}_{ns}") for ns in range(NSL)]
                for j in range(FCP)]
        tr_s = [[tpp.tile([P, 2, NF], E4, name=f"tr_{j}_{ns}",
                          tag=f"tr_{j}_{ns}") for ns in range(NSL)]
                for j in range(FCP)]
        p8_s = [[tpp.tile([P, 2, NF], E4, name=f"p8_{j}_{ns}",
                          tag=f"p8_{j}_{ns}") for ns in range(NSL)]
                for j in range(FCP)]
        pr_s = [[tpp.tile([P, 2, NF], E4, name=f"pr_{j}_{ns}",
                          tag=f"pr_{j}_{ns}") for ns in range(NSL)]
                for j in range(FCP)]

        # -------- Phase 1: projections (3-term fp8 DR) --------
        # psum = x8@W8' + x8@Wr' + xr8@W8'   (W' = 32W; 12 DR per group)
        # T = tanh(psum/64 + bt/2)  -> bf16 staging + e4m3 + residual
        # phi = psum/32 + bp        -> same
        for ns in range(NSL):
            for fc in range(FCH):
                fsl = slice(fc * P, (fc + 1) * P)
                ps = psum.tile([P, NF], F32, name="pst", tag="pst")
                nmm = 0
                for dcp in range(DCH // 2):
                    s2 = slice(2 * dcp, 2 * dcp + 2)
                    for (w_, x_) in ((wt8_s, xt8_s[ns]), (wtr_s, xt8_s[ns]),
                                     (wt8_s, xtr_s[ns])):
                        nc.tensor.matmul(ps, w_[:, s2, fsl], x_[:, s2],
                                         start=(nmm == 0), stop=(nmm == 11),
                                         perf_mode=DR)
                        nmm += 1
                tprec = tst.tile([P, NF], BF16, name="tpr", tag="tpr")
                nc.scalar.activation(tprec, ps, AF.Tanh,
                                     bias=bt2_s[:, fc:fc + 1], scale=1.0 / 64)
                nc.scalar.activation(t8_s[fc // 2][ns][:, fc % 2], ps, AF.Tanh,
                                     bias=bt2_s[:, fc:fc + 1], scale=1.0 / 64)
                nc.vector.tensor_sub(tr_s[fc // 2][ns][:, fc % 2], tprec,
                                     t8_s[fc // 2][ns][:, fc % 2])
            for fc in range(FCH):
                fsl = slice(fc * P, (fc + 1) * P)
                ps = psum.tile([P, NF], F32, name="pst", tag="pst")
                nmm = 0
                for dcp in range(DCH // 2):
                    s2 = slice(2 * dcp, 2 * dcp + 2)
                    for (w_, x_) in ((wp8_s, xt8_s[ns]), (wpr_s, xt8_s[ns]),
                                     (wp8_s, xtr_s[ns])):
                        nc.tensor.matmul(ps, w_[:, s2, fsl], x_[:, s2],
                                         start=(nmm == 0), stop=(nmm == 11),
                                         perf_mode=DR)
                        nmm += 1
                pprec = tst.tile([P, NF], BF16, name="ppr", tag="ppr")
                nc.vector.tensor_scalar(pprec, ps, 1.0 / WSCALE,
                                        bp_s[:, fc:fc + 1], ALU.mult, ALU.add)
                nc.scalar.activation(p8_s[fc // 2][ns][:, fc % 2], ps,
                                     AF.Identity, bias=bp_s[:, fc:fc + 1],
                                     scale=1.0 / WSCALE)
                nc.vector.tensor_sub(pr_s[fc // 2][ns][:, fc % 2], pprec,
                                     p8_s[fc // 2][ns][:, fc % 2])

        # phase-1 operands die here; reuse their space for phase-3 x tiles
        ph1cm.__exit__(None, None, None)
        xnp = es.enter_context(tc.tile_pool(name="xnp", bufs=1))
        xn8_s = xnp.tile([P, NCH, D], E4, name="xn8s", tag="xn8s")
        xnr_s = xnp.tile([P, NCH, D], E4, name="xnrs", tag="xnrs")
        nc.sync.dma_start(out=xn8_s, in_=xn8_d[:])
        nc.sync.dma_start(out=xnr_s, in_=xnr_d[:])

        # att8 (and Ar) per m-chunk-pair: phase-3 stationary APs span two
        # adjacent chunks; writes stream per chunk.
        a8_s = [apool.tile([P, 2, N], E4, name=f"a8_{j}", tag=f"a8_{j}")
                for j in range(NCH // 2)]
        if WEIGHTED_TERMS == 3:
            ar_s = [apool.tile([P, 2, N], E4, name=f"ar_{j}", tag=f"ar_{j}")
                    for j in range(NCH // 2)]

        # -------- Phase 2: scores + softmax --------
        # st[m, n] = T.phi + resid terms (= 2*logit - sum_f phi[m, f])
        # E = exp(0.5*st - 20) bf16 (+ row-sum via accum_out)
        # att8 = e4m3(E * recip)  [ACT Copy with per-partition scale]
        for mc in range(NCH):
            msl = slice(mc * P, (mc + 1) * P)
            sums = stats.tile([P, NSL], F32, name="sums", tag="sums")
            e_t = epool.tile([P, N], BF16, name="et", tag="et")
            for ns in range(NSL):
                nsl = slice(ns * NF, (ns + 1) * NF)
                ps = psum.tile([P, NF], F32, name="pst", tag="pst")
                nmm = 0
                for j in range(FCP):
                    for (sta, mov) in ((p8_s, t8_s), (pr_s, t8_s),
                                       (p8_s, tr_s)):
                        nc.tensor.matmul(
                            ps, sta[j][mc // 4][:, :, (mc % 4) * P:
                                                (mc % 4 + 1) * P],
                            mov[j][ns],
                            start=(nmm == 0), stop=(nmm == 5), perf_mode=DR)
                        nmm += 1
                nc.scalar.activation(e_t[:, nsl], ps, AF.Exp, bias=eb_s,
                                     scale=0.5,
                                     accum_out=sums[:, ns:ns + 1])
            rs = stats.tile([P, 1], F32, name="rs", tag="rs")
            nc.vector.reduce_sum(rs, sums, axis=AX)
            rc = stats.tile([P, 1], F32, name="rc", tag="rc")
            nc.vector.reciprocal(rc, rs)
            nc.scalar.activation(a8_s[mc // 2][:, mc % 2], e_t, AF.Copy,
                                 scale=rc)
            if WEIGHTED_TERMS == 3:
                apre = appool.tile([P, N], BF16, name="ap", tag="ap")
                nc.vector.tensor_scalar_mul(apre, e_t, rc)
                nc.vector.tensor_sub(ar_s[mc // 2][:, mc % 2], apre,
                                     a8_s[mc // 2][:, mc % 2])

        # -------- Phase 3: weighted sum + residual --------
        # out[n, d] = sum_m att[m, n] * x[m, d] + x[n, d]
        for nch in range(NCH):
            nsl128 = slice(nch * P, (nch + 1) * P)
            xrt = xrp.tile([P, D], BF16, name="xrt", tag="xrt")
            nc.sync.dma_start(out=xrt, in_=xr_d[nsl128, :])
            osb = outp.tile([P, D], F32, name="osb", tag="osb")
            for dsl in range(DSL):
                dslc = slice(dsl * NF, (dsl + 1) * NF)
                ps = psum.tile([P, NF], F32, name="pst", tag="pst")
                nterm = WEIGHTED_TERMS
                nmm = 0
                for gp in range(NCH // 2):
                    g2 = slice(2 * gp, 2 * gp + 2)
                    pairs = [(a8_s[gp], xn8_s[:, g2, dslc]),
                             (a8_s[gp], xnr_s[:, g2, dslc])]
                    if nterm == 3:
                        pairs.append((ar_s[gp], xn8_s[:, g2, dslc]))
                    for (sta, mov) in pairs:
                        nc.tensor.matmul(ps, sta[:, :, nsl128], mov,
                                         start=(nmm == 0),
                                         stop=(nmm == 8 * nterm - 1),
                                         perf_mode=DR)
                        nmm += 1
                nc.vector.tensor_add(osb[:, dslc], ps, xrt[:, dslc])
            nc.sync.dma_start(out=out_d[nsl128, :], in_=osb)
        es.close()
    nc.finalize()  # Bacc legalization passes (wait splitting, reg alloc, ...)
    return nc


_NC = None


def _get_nc():
    global _NC
    if _NC is None:
        _NC = build_bass()
    return _NC


def _e4(a):
    return np.asarray(a, np.float32).astype(E4NP)


def make_in_maps(x, Wt, bt, Wp, bp):
    def wswz(w):
        # [D, F] -> [P, DCH, F]: [p, dc, f] = w[dc*128+p, f]
        return np.ascontiguousarray(
            w.reshape(DCH, P, F).transpose(1, 0, 2))

    def split_w(W):
        wp = WSCALE * np.asarray(W, np.float32)
        w8 = _e4(wp)
        wr = _e4(wp - w8.astype(np.float32))
        return wswz(w8), wswz(wr)

    wt8, wtr8 = split_w(Wt)
    wp8, wpr8 = split_w(Wp)
    fch = bt.size // P
    bt2 = np.ascontiguousarray(
        (np.asarray(bt, np.float32) / 2).reshape(fch, P).T)
    bp_r = np.ascontiguousarray(np.asarray(bp, np.float32).reshape(fch, P).T)

    common = {"wt8": wt8, "wtr8": wtr8, "wp8": wp8, "wpr8": wpr8,
              "bt2": bt2, "bp": bp_r}

    def xtimg(a):  # [N, D] e4m3 -> per-ns [P, DCH, NF] images of a.T
        at = np.ascontiguousarray(a.T)         # [D, N]
        r = at.reshape(DCH, P, N)
        return [np.ascontiguousarray(r[:, :, ns * NF:(ns + 1) * NF]
                                     .transpose(1, 0, 2))
                for ns in range(NSL)]

    def xnimg(a):  # [N, D] e4m3 -> [P, NCH, D]
        return np.ascontiguousarray(
            a.reshape(NCH, P, D).transpose(1, 0, 2))

    in_maps = []
    for b in range(x.shape[0]):
        xb = np.ascontiguousarray(np.asarray(x[b], np.float32))
        x8 = _e4(xb)
        xr8 = _e4(xb - x8.astype(np.float32))
        m = dict(common)
        for ns, img in enumerate(xtimg(x8)):
            m[f"xt8{ns}"] = img
        for ns, img in enumerate(xtimg(xr8)):
            m[f"xtr8{ns}"] = img
        m["xn8"] = xnimg(x8)
        m["xnr8"] = xnimg(xr8)
        m["xr"] = xb.astype(ml_dtypes.bfloat16)
        in_maps.append(m)
    return in_maps


def run(inputs, trace=False):
    """Run on 8 NeuronCores; returns (out [B,N,D] f32, BassKernelResults)."""
    x = inputs["x"]
    assert x.shape == (B, N, D), x.shape
    nc = _get_nc()
    in_maps = make_in_maps(x, inputs["Wt"], inputs["bt"], inputs["Wp"],
                           inputs["bp"])
    res = run_bass_kernel_spmd(nc, in_maps, core_ids=list(range(B)),
                               trace=trace)
    out = np.stack([res.results[c]["out"] for c in range(B)], axis=0)
    return out.astype(np.float32), res


def kernel(**inputs) -> np.ndarray:
    out, _ = run(inputs)
    return out


# revision 7
# speedup vs baseline: 1.4410x; 1.1538x over previous
"""Trainium2 Bass kernel for nn_AttentionModule (dense_transformer), fp8 DR.

Reference computation (per batch sample b):
    theta = sigmoid(x @ Wt + bt)            # [N, F]
    phi   = x @ Wp + bp                     # [N, F]
    att   = theta @ phi.T                   # [N(n), N(m)]
    att   = softmax(att, axis over n)       # softmax over QUERY axis
    out   = att(n,m) @ x(m,d) + x           # [N, D]

Strategy: pure data parallelism (B=8 samples, one per core, no
collectives) + fp8-e4m3 DoubleRow matmuls (0.5 cycles/row, 2x128
contraction per instruction = 4x bf16 MAC rate in the cost model).

Precision scheme (validated in numerics2.py, rel-l2 vs f64 ref):
 - every bf16 matmul X@W is replaced by 2-3 fp8 terms
       X8@W8 + X8@Wr + Xr@W8      (r = unscaled e4m3 residual)
   accumulated in the same fp32 PSUM group; residual pairs restore
   ~bf16-equivalent precision at 0.5-0.75x of bf16 matmul time.
 - weights are pre-scaled by 32 on host (W' = 32W) so their e4m3
   residuals land in normal range; the 1/32 is folded into the
   activation scale.
 - theta is carried as T = tanh(z/2) = 2*sigmoid(z)-1 in (-1,1):
   the scores logit becomes 0.5*(T.phi) + 0.5*sum_f(phi[m]); the
   second term is constant per m and cancels in the softmax over n
   (exp bias -20; measured exponent range on real inputs [-8, 11.1]).
 - att is quantized to e4m3 AFTER normalization; weighted uses
   2 terms (att8@x8 + att8@xr8, rel~1.38e-2) or 3 terms (+Ar@x8,
   rel~5.5e-3) per WEIGHTED_TERMS.

Scheduling:
 - PE work (cost model, full clock): proj 41us + scores 41us +
   weighted 55us (2t) = 137us vs 218us bf16 roofline.
 - exp runs on ACT with accum_out giving row-sums for free; att8
   normalization on ACT (Copy with per-partition scale=recip).
 - tiles are split to consumer granularity (deps are tile-granular):
   T/P operands per (fc-pair, ns-512), att8/Ar per m-chunk-pair.
 - SBUF: phase-1 operand pool (48KB) is closed after phase 1 and its
   space reused for the phase-3 x tiles (LIFO pool stack); E lives in
   a rotating 4-deep pool (each E[mc] dies once att8[mc] is built).
"""

import numpy as np
import ml_dtypes

import concourse.bass as bass
import concourse.bacc as bacc
import concourse.mybir as mybir
from concourse.tile import TileContext
from concourse.bass_utils import run_bass_kernel_spmd

P = 128
B, N, D, F = 8, 2048, 1024, 512
NCH = N // P    # 16 m/n chunks
DCH = D // P    # 8 d chunks
FCH = F // P    # 4 f chunks
NF = 512        # psum moving width (one fp32 bank)
NSL = N // NF   # 4 column slices
DSL = D // NF   # 2 output d slices
WSCALE = 32.0   # host weight pre-scale (residuals out of denormals)

WEIGHTED_TERMS = 2   # 2: att8@(x8+xr8) ~1.4e-2 | 3: +Ar@x8 ~5.5e-3

BF16 = mybir.dt.bfloat16
F32 = mybir.dt.float32
E4 = mybir.dt.float8e4
AX = mybir.AxisListType.X
AF = mybir.ActivationFunctionType
DR = mybir.MatmulPerfMode.DoubleRow
ALU = mybir.AluOpType
E4NP = ml_dtypes.float8_e4m3


def build_bass():
    nc = bacc.Bacc()

    wt8_d = nc.declare_dram_parameter("wt8", [P, DCH, F], E4, isOutput=False)
    wtr_d = nc.declare_dram_parameter("wtr8", [P, DCH, F], E4, isOutput=False)
    wp8_d = nc.declare_dram_parameter("wp8", [P, DCH, F], E4, isOutput=False)
    wpr_d = nc.declare_dram_parameter("wpr8", [P, DCH, F], E4, isOutput=False)
    bt2_d = nc.declare_dram_parameter("bt2", [P, FCH], F32, isOutput=False)
    bp_d = nc.declare_dram_parameter("bp", [P, FCH], F32, isOutput=False)
    xt8_d = [nc.declare_dram_parameter(f"xt8{ns}", [P, DCH, NF], E4,
                                       isOutput=False) for ns in range(NSL)]
    xtr_d = [nc.declare_dram_parameter(f"xtr8{ns}", [P, DCH, NF], E4,
                                       isOutput=False) for ns in range(NSL)]
    xn8_d = nc.declare_dram_parameter("xn8", [P, NCH, D], E4, isOutput=False)
    xnr_d = nc.declare_dram_parameter("xnr8", [P, NCH, D], E4, isOutput=False)
    xr_d = nc.declare_dram_parameter("xr", [N, D], BF16, isOutput=False)
    out_d = nc.declare_dram_parameter("out", [N, D], F32, isOutput=True)

    with TileContext(nc) as tc:
        from contextlib import ExitStack
        es = ExitStack()
        cpool = es.enter_context(tc.tile_pool(name="const", bufs=1))
        stats = es.enter_context(tc.tile_pool(name="stats", bufs=4))
        epool = es.enter_context(tc.tile_pool(name="ep", bufs=4))
        apool = es.enter_context(tc.tile_pool(name="a8", bufs=1))
        appool = es.enter_context(tc.tile_pool(name="apre", bufs=2))
        xrp = es.enter_context(tc.tile_pool(name="xrp", bufs=3))
        outp = es.enter_context(tc.tile_pool(name="outp", bufs=3))
        tpp = es.enter_context(tc.tile_pool(name="tpp", bufs=1))
        tst = es.enter_context(tc.tile_pool(name="tst", bufs=3))
        psum = es.enter_context(tc.tile_pool(name="psum", bufs=8, space="PSUM"))
        ph1cm = tc.tile_pool(name="ph1", bufs=1)
        ph1 = ph1cm.__enter__()

        # ---- constants / stats ----
        bt2_s = cpool.tile([P, FCH], F32, name="bt2s", tag="bt2s")
        bp_s = cpool.tile([P, FCH], F32, name="bps", tag="bps")
        zx = cpool.tile([P, NF], BF16, name="zx", tag="zx")
        nc.vector.memset(zx, 0)
        eb_s = cpool.tile([P, 1], F32, name="ebs", tag="ebs")
        nc.vector.memset(eb_s, -20.0)

        # PE warm-up: the pstate ramp holds PE below 2.4GHz for ~3.4us of
        # sustained activity; the first real matmul waits on DMA anyway, so
        # burn the idle time on dummy matmuls (costless: PE was idle).
        NWARM = 20
        zp = psum.tile([P, NF], F32, name="pst", tag="pst")
        for i in range(NWARM):
            nc.tensor.matmul(zp, zx[:, 0:P], zx, start=(i == 0),
                             stop=(i == NWARM - 1))

        # ---- phase-1 operand tiles + DMAs (first-use order) ----
        wt8_s = ph1.tile([P, DCH, F], E4, name="wt8s", tag="wt8s")
        wtr_s = ph1.tile([P, DCH, F], E4, name="wtrs", tag="wtrs")
        wp8_s = ph1.tile([P, DCH, F], E4, name="wp8s", tag="wp8s")
        wpr_s = ph1.tile([P, DCH, F], E4, name="wprs", tag="wprs")
        xt8_s = [ph1.tile([P, DCH, NF], E4, name=f"xt8{ns}", tag=f"xt8{ns}")
                 for ns in range(NSL)]
        xtr_s = [ph1.tile([P, DCH, NF], E4, name=f"xtr{ns}", tag=f"xtr{ns}")
                 for ns in range(NSL)]

        nc.sync.dma_start(out=wt8_s, in_=wt8_d[:])
        nc.sync.dma_start(out=xt8_s[0], in_=xt8_d[0][:])
        nc.sync.dma_start(out=bt2_s, in_=bt2_d[:])
        nc.sync.dma_start(out=wtr_s, in_=wtr_d[:])
        nc.sync.dma_start(out=xtr_s[0], in_=xtr_d[0][:])
        nc.sync.dma_start(out=wp8_s, in_=wp8_d[:])
        nc.sync.dma_start(out=wpr_s, in_=wpr_d[:])
        nc.sync.dma_start(out=bp_s, in_=bp_d[:])
        for ns in range(1, NSL):
            nc.sync.dma_start(out=xt8_s[ns], in_=xt8_d[ns][:])
            nc.sync.dma_start(out=xtr_s[ns], in_=xtr_d[ns][:])

        # T/P operand tiles: per (fc-pair, ns-512) so scores(mc, ns) only
        # waits on the exact phase-1 blocks it reads.
        FCP = FCH // 2
        t8_s = [[tpp.tile([P, 2, NF], E4, name=f"t8_{j}_{ns}",
                          tag=f"t8_{j}_{ns}") for ns in range(NSL)]
                for j in range(FCP)]
        tr_s = [[tpp.tile([P, 2, NF], E4, name=f"tr_{j}_{ns}",
                          tag=f"tr_{j}_{ns}") for ns in range(NSL)]
                for j in range(FCP)]
        p8_s = [[tpp.tile([P, 2, NF], E4, name=f"p8_{j}_{ns}",
                          tag=f"p8_{j}_{ns}") for ns in range(NSL)]
                for j in range(FCP)]
        pr_s = [[tpp.tile([P, 2, NF], E4, name=f"pr_{j}_{ns}",
                          tag=f"pr_{j}_{ns}") for ns in range(NSL)]
                for j in range(FCP)]

        # -------- Phase 1: projections (3-term fp8 DR) --------
        # psum = x8@W8' + x8@Wr' + xr8@W8'   (W' = 32W; 12 DR per group)
        # T = tanh(psum/64 + bt/2)  -> bf16 staging + e4m3 + residual
        # phi = psum/32 + bp        -> same
        for ns in range(NSL):
            for fc in range(FCH):
                fsl = slice(fc * P, (fc + 1) * P)
                ps = psum.tile([P, NF], F32, name="pst", tag="pst")
                nmm = 0
                for dcp in range(DCH // 2):
                    s2 = slice(2 * dcp, 2 * dcp + 2)
                    for (w_, x_) in ((wt8_s, xt8_s[ns]), (wtr_s, xt8_s[ns]),
                                     (wt8_s, xtr_s[ns])):
                        nc.tensor.matmul(ps, w_[:, s2, fsl], x_[:, s2],
                                         start=(nmm == 0), stop=(nmm == 11),
                                         perf_mode=DR)
                        nmm += 1
                tprec = tst.tile([P, NF], BF16, name="tpr", tag="tpr")
                nc.scalar.activation(tprec, ps, AF.Tanh,
                                     bias=bt2_s[:, fc:fc + 1], scale=1.0 / 64)
                nc.scalar.activation(t8_s[fc // 2][ns][:, fc % 2], ps, AF.Tanh,
                                     bias=bt2_s[:, fc:fc + 1], scale=1.0 / 64)
                nc.vector.tensor_sub(tr_s[fc // 2][ns][:, fc % 2], tprec,
                                     t8_s[fc // 2][ns][:, fc % 2])
            for fc in range(FCH):
                fsl = slice(fc * P, (fc + 1) * P)
                ps = psum.tile([P, NF], F32, name="pst", tag="pst")
                nmm = 0
                for dcp in range(DCH // 2):
                    s2 = slice(2 * dcp, 2 * dcp + 2)
                    for (w_, x_) in ((wp8_s, xt8_s[ns]), (wpr_s, xt8_s[ns]),
                                     (wp8_s, xtr_s[ns])):
                        nc.tensor.matmul(ps, w_[:, s2, fsl], x_[:, s2],
                                         start=(nmm == 0), stop=(nmm == 11),
                                         perf_mode=DR)
                        nmm += 1
                pprec = tst.tile([P, NF], BF16, name="ppr", tag="ppr")
                nc.vector.tensor_scalar(pprec, ps, 1.0 / WSCALE,
                                        bp_s[:, fc:fc + 1], ALU.mult, ALU.add)
                nc.scalar.activation(p8_s[fc // 2][ns][:, fc % 2], ps,
                                     AF.Identity, bias=bp_s[:, fc:fc + 1],
                                     scale=1.0 / WSCALE)
                nc.vector.tensor_sub(pr_s[fc // 2][ns][:, fc % 2], pprec,
                                     p8_s[fc // 2][ns][:, fc % 2])

        # phase-1 operands die here; reuse their space for phase-3 x tiles
        ph1cm.__exit__(None, None, None)
        xnp = es.enter_context(tc.tile_pool(name="xnp", bufs=1))
        xn8_s = xnp.tile([P, NCH, D], E4, name="xn8s", tag="xn8s")
        xnr_s = xnp.tile([P, NCH, D], E4, name="xnrs", tag="xnrs")
        nc.sync.dma_start(out=xn8_s, in_=xn8_d[:])
        nc.sync.dma_start(out=xnr_s, in_=xnr_d[:])

        # att8 (and Ar) per m-chunk-pair: phase-3 stationary APs span two
        # adjacent chunks; writes stream per chunk.
        a8_s = [apool.tile([P, 2, N], E4, name=f"a8_{j}", tag=f"a8_{j}")
                for j in range(NCH // 2)]
        if WEIGHTED_TERMS == 3:
            ar_s = [apool.tile([P, 2, N], E4, name=f"ar_{j}", tag=f"ar_{j}")
                    for j in range(NCH // 2)]

        # -------- Phase 2: scores + softmax --------
        # st[m, n] = T.phi + resid terms (= 2*logit - sum_f phi[m, f])
        # E = exp(0.5*st - 20) bf16 (+ row-sum via accum_out)
        # att8 = e4m3(E * recip)  [ACT Copy with per-partition scale]
        for mc in range(NCH):
            msl = slice(mc * P, (mc + 1) * P)
            sums = stats.tile([P, NSL], F32, name="sums", tag="sums")
            e_t = epool.tile([P, N], BF16, name="et", tag="et")
            for ns in range(NSL):
                nsl = slice(ns * NF, (ns + 1) * NF)
                ps = psum.tile([P, NF], F32, name="pst", tag="pst")
                nmm = 0
                for j in range(FCP):
                    for (sta, mov) in ((p8_s, t8_s), (pr_s, t8_s),
                                       (p8_s, tr_s)):
                        nc.tensor.matmul(
                            ps, sta[j][mc // 4][:, :, (mc % 4) * P:
                                                (mc % 4 + 1) * P],
                            mov[j][ns],
                            start=(nmm == 0), stop=(nmm == 5), perf_mode=DR)
                        nmm += 1
                nc.scalar.activation(e_t[:, nsl], ps, AF.Exp, bias=eb_s,
                                     scale=0.5,
                                     accum_out=sums[:, ns:ns + 1])
            rs = stats.tile([P, 1], F32, name="rs", tag="rs")
            nc.vector.reduce_sum(rs, sums, axis=AX)
            rc = stats.tile([P, 1], F32, name="rc", tag="rc")
            nc.vector.reciprocal(rc, rs)
            # normalization on DVE: ACT is busy with the exp stream, and the
            # per-chunk softmax chain otherwise paces phase 2 above PE rate
            nc.vector.tensor_scalar_mul(a8_s[mc // 2][:, mc % 2], e_t, rc)
            if WEIGHTED_TERMS == 3:
                apre = appool.tile([P, N], BF16, name="ap", tag="ap")
                nc.vector.tensor_scalar_mul(apre, e_t, rc)
                nc.vector.tensor_sub(ar_s[mc // 2][:, mc % 2], apre,
                                     a8_s[mc // 2][:, mc % 2])

        # -------- Phase 3: weighted sum + residual --------
        # out[n, d] = sum_m att[m, n] * x[m, d] + x[n, d]
        for nch in range(NCH):
            nsl128 = slice(nch * P, (nch + 1) * P)
            xrt = xrp.tile([P, D], BF16, name="xrt", tag="xrt")
            nc.sync.dma_start(out=xrt, in_=xr_d[nsl128, :])
            osb = outp.tile([P, D], F32, name="osb", tag="osb")
            for dsl in range(DSL):
                dslc = slice(dsl * NF, (dsl + 1) * NF)
                ps = psum.tile([P, NF], F32, name="pst", tag="pst")
                nterm = WEIGHTED_TERMS
                nmm = 0
                for gp in range(NCH // 2):
                    g2 = slice(2 * gp, 2 * gp + 2)
                    pairs = [(a8_s[gp], xn8_s[:, g2, dslc]),
                             (a8_s[gp], xnr_s[:, g2, dslc])]
                    if nterm == 3:
                        pairs.append((ar_s[gp], xn8_s[:, g2, dslc]))
                    for (sta, mov) in pairs:
                        nc.tensor.matmul(ps, sta[:, :, nsl128], mov,
                                         start=(nmm == 0),
                                         stop=(nmm == 8 * nterm - 1),
                                         perf_mode=DR)
                        nmm += 1
                nc.vector.tensor_add(osb[:, dslc], ps, xrt[:, dslc])
                if nch == NCH - 1:
                    # tail: ship each half as soon as its add lands
                    nc.sync.dma_start(out=out_d[nsl128, dslc],
                                      in_=osb[:, dslc])
            if nch < NCH - 1:
                nc.sync.dma_start(out=out_d[nsl128, :], in_=osb)
        es.close()
    nc.finalize()  # Bacc legalization passes (wait splitting, reg alloc, ...)
    return nc


_NC = None


def _get_nc():
    global _NC
    if _NC is None:
        _NC = build_bass()
    return _NC


def _e4(a):
    return np.asarray(a, np.float32).astype(E4NP)


def make_in_maps(x, Wt, bt, Wp, bp):
    def wswz(w):
        # [D, F] -> [P, DCH, F]: [p, dc, f] = w[dc*128+p, f]
        return np.ascontiguousarray(
            w.reshape(DCH, P, F).transpose(1, 0, 2))

    def split_w(W):
        wp = WSCALE * np.asarray(W, np.float32)
        w8 = _e4(wp)
        wr = _e4(wp - w8.astype(np.float32))
        return wswz(w8), wswz(wr)

    wt8, wtr8 = split_w(Wt)
    wp8, wpr8 = split_w(Wp)
    fch = bt.size // P
    bt2 = np.ascontiguousarray(
        (np.asarray(bt, np.float32) / 2).reshape(fch, P).T)
    bp_r = np.ascontiguousarray(np.asarray(bp, np.float32).reshape(fch, P).T)

    common = {"wt8": wt8, "wtr8": wtr8, "wp8": wp8, "wpr8": wpr8,
              "bt2": bt2, "bp": bp_r}

    def xtimg(a):  # [N, D] e4m3 -> per-ns [P, DCH, NF] images of a.T
        at = np.ascontiguousarray(a.T)         # [D, N]
        r = at.reshape(DCH, P, N)
        return [np.ascontiguousarray(r[:, :, ns * NF:(ns + 1) * NF]
                                     .transpose(1, 0, 2))
                for ns in range(NSL)]

    def xnimg(a):  # [N, D] e4m3 -> [P, NCH, D]
        return np.ascontiguousarray(
            a.reshape(NCH, P, D).transpose(1, 0, 2))

    in_maps = []
    for b in range(x.shape[0]):
        xb = np.ascontiguousarray(np.asarray(x[b], np.float32))
        x8 = _e4(xb)
        xr8 = _e4(xb - x8.astype(np.float32))
        m = dict(common)
        for ns, img in enumerate(xtimg(x8)):
            m[f"xt8{ns}"] = img
        for ns, img in enumerate(xtimg(xr8)):
            m[f"xtr8{ns}"] = img
        m["xn8"] = xnimg(x8)
        m["xnr8"] = xnimg(xr8)
        m["xr"] = xb.astype(ml_dtypes.bfloat16)
        in_maps.append(m)
    return in_maps


def run(inputs, trace=False):
    """Run on 8 NeuronCores; returns (out [B,N,D] f32, BassKernelResults)."""
    x = inputs["x"]
    assert x.shape == (B, N, D), x.shape
    nc = _get_nc()
    in_maps = make_in_maps(x, inputs["Wt"], inputs["bt"], inputs["Wp"],
                           inputs["bp"])
    res = run_bass_kernel_spmd(nc, in_maps, core_ids=list(range(B)),
                               trace=trace)
    out = np.stack([res.results[c]["out"] for c in range(B)], axis=0)
    return out.astype(np.float32), res


def kernel(**inputs) -> np.ndarray:
    out, _ = run(inputs)
    return out


# revision 11
# speedup vs baseline: 1.4650x; 1.0167x over previous
"""Trainium2 Bass kernel for nn_AttentionModule (dense_transformer), fp8 DR.

Reference computation (per batch sample b):
    theta = sigmoid(x @ Wt + bt)            # [N, F]
    phi   = x @ Wp + bp                     # [N, F]
    att   = theta @ phi.T                   # [N(n), N(m)]
    att   = softmax(att, axis over n)       # softmax over QUERY axis
    out   = att(n,m) @ x(m,d) + x           # [N, D]

Strategy: pure data parallelism (B=8 samples, one per core, no
collectives) + fp8-e4m3 DoubleRow matmuls (0.5 cycles/row, 2x128
contraction per instruction = 4x bf16 MAC rate in the cost model).

Precision scheme (validated in numerics2.py, rel-l2 vs f64 ref):
 - every bf16 matmul X@W is replaced by 2-3 fp8 terms
       X8@W8 + X8@Wr + Xr@W8      (r = unscaled e4m3 residual)
   accumulated in the same fp32 PSUM group; residual pairs restore
   ~bf16-equivalent precision at 0.5-0.75x of bf16 matmul time.
 - weights are pre-scaled by 32 on host (W' = 32W) so their e4m3
   residuals land in normal range; the 1/32 is folded into the
   activation scale.
 - theta is carried as T = tanh(z/2) = 2*sigmoid(z)-1 in (-1,1):
   the scores logit becomes 0.5*(T.phi) + 0.5*sum_f(phi[m]); the
   second term is constant per m and cancels in the softmax over n
   (exp bias -20; measured exponent range on real inputs [-8, 11.1]).
 - att is quantized to e4m3 AFTER normalization; weighted uses
   2 terms (att8@x8 + att8@xr8, rel~1.38e-2) or 3 terms (+Ar@x8,
   rel~5.5e-3) per WEIGHTED_TERMS.

Scheduling:
 - PE work (cost model, full clock): proj 41us + scores 41us +
   weighted 55us (2t) = 137us vs 218us bf16 roofline.
 - exp runs on ACT with accum_out giving row-sums for free; att8
   normalization on ACT (Copy with per-partition scale=recip).
 - tiles are split to consumer granularity (deps are tile-granular):
   T/P operands per (fc-pair, ns-512), att8/Ar per m-chunk-pair.
 - SBUF: phase-1 operand pool (48KB) is closed after phase 1 and its
   space reused for the phase-3 x tiles (LIFO pool stack); E lives in
   a rotating 4-deep pool (each E[mc] dies once att8[mc] is built).
"""

import numpy as np
import ml_dtypes

import concourse.bass as bass
import concourse.bacc as bacc
import concourse.mybir as mybir
from concourse.tile import TileContext
from concourse.bass_utils import run_bass_kernel_spmd

P = 128
B, N, D, F = 8, 2048, 1024, 512
NCH = N // P    # 16 m/n chunks
DCH = D // P    # 8 d chunks
FCH = F // P    # 4 f chunks
NF = 512        # psum moving width (one fp32 bank)
NSL = N // NF   # 4 column slices
DSL = D // NF   # 2 output d slices
WSCALE = 32.0   # host weight pre-scale (residuals out of denormals)

WEIGHTED_TERMS = 2   # 2: att8@(x8+xr8) ~1.4e-2 | 3: +Ar@x8 ~5.5e-3

BF16 = mybir.dt.bfloat16
F32 = mybir.dt.float32
E4 = mybir.dt.float8e4
AX = mybir.AxisListType.X
AF = mybir.ActivationFunctionType
DR = mybir.MatmulPerfMode.DoubleRow
ALU = mybir.AluOpType
E4NP = ml_dtypes.float8_e4m3


def build_bass():
    nc = bacc.Bacc()

    wt8_d = nc.declare_dram_parameter("wt8", [P, DCH, F], E4, isOutput=False)
    wtr_d = nc.declare_dram_parameter("wtr8", [P, DCH, F], E4, isOutput=False)
    wp8_d = nc.declare_dram_parameter("wp8", [P, DCH, F], E4, isOutput=False)
    wpr_d = nc.declare_dram_parameter("wpr8", [P, DCH, F], E4, isOutput=False)
    bt2_d = nc.declare_dram_parameter("bt2", [P, FCH], F32, isOutput=False)
    bp_d = nc.declare_dram_parameter("bp", [P, FCH], F32, isOutput=False)
    xt8_d = [nc.declare_dram_parameter(f"xt8{ns}", [P, DCH, NF], E4,
                                       isOutput=False) for ns in range(NSL)]
    xtr_d = [nc.declare_dram_parameter(f"xtr8{ns}", [P, DCH, NF], E4,
                                       isOutput=False) for ns in range(NSL)]
    xn8_d = nc.declare_dram_parameter("xn8", [P, NCH, D], E4, isOutput=False)
    xnr_d = nc.declare_dram_parameter("xnr8", [P, NCH, D], E4, isOutput=False)
    xr_d = nc.declare_dram_parameter("xr", [N, D], BF16, isOutput=False)
    out_d = nc.declare_dram_parameter("out", [N, D], F32, isOutput=True)

    with TileContext(nc) as tc:
        from contextlib import ExitStack
        es = ExitStack()
        cpool = es.enter_context(tc.tile_pool(name="const", bufs=1))
        stats = es.enter_context(tc.tile_pool(name="stats", bufs=8))
        epool = es.enter_context(tc.tile_pool(name="ep", bufs=8))
        apool = es.enter_context(tc.tile_pool(name="a8", bufs=1))
        appool = es.enter_context(tc.tile_pool(name="apre", bufs=2))
        xrp = es.enter_context(tc.tile_pool(name="xrp", bufs=3))
        outp = es.enter_context(tc.tile_pool(name="outp", bufs=3))
        tpp = es.enter_context(tc.tile_pool(name="tpp", bufs=1))
        tst = es.enter_context(tc.tile_pool(name="tst", bufs=3))
        psum = es.enter_context(tc.tile_pool(name="psum", bufs=8, space="PSUM"))
        ph1cm = tc.tile_pool(name="ph1", bufs=1)
        ph1 = ph1cm.__enter__()

        # ---- constants / stats ----
        bt2_s = cpool.tile([P, FCH], F32, name="bt2s", tag="bt2s")
        bp_s = cpool.tile([P, FCH], F32, name="bps", tag="bps")
        zx = cpool.tile([P, P], BF16, name="zx", tag="zx")
        nc.vector.memset(zx, 0)
        eb_s = cpool.tile([P, 1], F32, name="ebs", tag="ebs")
        nc.vector.memset(eb_s, -20.0)

        # PE warm-up: the pstate ramp holds PE below 2.4GHz for ~3.4us of
        # sustained activity; the first real matmul waits on DMA anyway, so
        # burn the idle time on dummy matmuls (costless: PE was idle).
        NWARM = 72  # 128-wide dummies: cover ~7us of startup DMA latency
        zp = psum.tile([P, NF], F32, name="pst", tag="pst")
        for i in range(NWARM):
            nc.tensor.matmul(zp[:, 0:P], zx, zx, start=(i == 0),
                             stop=(i == NWARM - 1))

        # ---- phase-1 operand tiles + DMAs (first-use order) ----
        wt8_s = ph1.tile([P, DCH, F], E4, name="wt8s", tag="wt8s")
        wtr_s = ph1.tile([P, DCH, F], E4, name="wtrs", tag="wtrs")
        wp8_s = ph1.tile([P, DCH, F], E4, name="wp8s", tag="wp8s")
        wpr_s = ph1.tile([P, DCH, F], E4, name="wprs", tag="wprs")
        xt8_s = [ph1.tile([P, DCH, NF], E4, name=f"xt8{ns}", tag=f"xt8{ns}")
                 for ns in range(NSL)]
        xtr_s = [ph1.tile([P, DCH, NF], E4, name=f"xtr{ns}", tag=f"xtr{ns}")
                 for ns in range(NSL)]

        nc.sync.dma_start(out=wt8_s, in_=wt8_d[:])
        nc.sync.dma_start(out=xt8_s[0], in_=xt8_d[0][:])
        nc.sync.dma_start(out=bt2_s, in_=bt2_d[:])
        nc.sync.dma_start(out=wtr_s, in_=wtr_d[:])
        nc.sync.dma_start(out=xtr_s[0], in_=xtr_d[0][:])
        nc.sync.dma_start(out=wp8_s, in_=wp8_d[:])
        nc.sync.dma_start(out=wpr_s, in_=wpr_d[:])
        nc.sync.dma_start(out=bp_s, in_=bp_d[:])
        for ns in range(1, NSL):
            nc.sync.dma_start(out=xt8_s[ns], in_=xt8_d[ns][:])
            nc.sync.dma_start(out=xtr_s[ns], in_=xtr_d[ns][:])

        # T/P operand tiles: per (fc-pair, ns-512) so scores(mc, ns) only
        # waits on the exact phase-1 blocks it reads.
        FCP = FCH // 2
        t8_s = [[tpp.tile([P, 2, NF], E4, name=f"t8_{j}_{ns}",
                          tag=f"t8_{j}_{ns}") for ns in range(NSL)]
                for j in range(FCP)]
        tr_s = [[tpp.tile([P, 2, NF], E4, name=f"tr_{j}_{ns}",
                          tag=f"tr_{j}_{ns}") for ns in range(NSL)]
                for j in range(FCP)]
        p8_s = [[tpp.tile([P, 2, NF], E4, name=f"p8_{j}_{ns}",
                          tag=f"p8_{j}_{ns}") for ns in range(NSL)]
                for j in range(FCP)]
        pr_s = [[tpp.tile([P, 2, NF], E4, name=f"pr_{j}_{ns}",
                          tag=f"pr_{j}_{ns}") for ns in range(NSL)]
                for j in range(FCP)]

        # -------- Phase 1: projections (3-term fp8 DR) --------
        # psum = x8@W8' + x8@Wr' + xr8@W8'   (W' = 32W; 12 DR per group)
        # T = tanh(psum/64 + bt/2)  -> bf16 staging + e4m3 + residual
        # phi = psum/32 + bp        -> same
        for ns in range(NSL):
            for fc in range(FCH):
                fsl = slice(fc * P, (fc + 1) * P)
                ps = psum.tile([P, NF], F32, name="pst", tag="pst")
                nmm = 0
                for dcp in range(DCH // 2):
                    s2 = slice(2 * dcp, 2 * dcp + 2)
                    for (w_, x_) in ((wt8_s, xt8_s[ns]), (wtr_s, xt8_s[ns]),
                                     (wt8_s, xtr_s[ns])):
                        nc.tensor.matmul(ps, w_[:, s2, fsl], x_[:, s2],
                                         start=(nmm == 0), stop=(nmm == 11),
                                         perf_mode=DR)
                        nmm += 1
                tprec = tst.tile([P, NF], BF16, name="tpr", tag="tpr")
                nc.scalar.activation(tprec, ps, AF.Tanh,
                                     bias=bt2_s[:, fc:fc + 1], scale=1.0 / 64)
                nc.scalar.activation(t8_s[fc // 2][ns][:, fc % 2], ps, AF.Tanh,
                                     bias=bt2_s[:, fc:fc + 1], scale=1.0 / 64)
                nc.vector.tensor_sub(tr_s[fc // 2][ns][:, fc % 2], tprec,
                                     t8_s[fc // 2][ns][:, fc % 2])
            for fc in range(FCH):
                fsl = slice(fc * P, (fc + 1) * P)
                ps = psum.tile([P, NF], F32, name="pst", tag="pst")
                nmm = 0
                for dcp in range(DCH // 2):
                    s2 = slice(2 * dcp, 2 * dcp + 2)
                    for (w_, x_) in ((wp8_s, xt8_s[ns]), (wpr_s, xt8_s[ns]),
                                     (wp8_s, xtr_s[ns])):
                        nc.tensor.matmul(ps, w_[:, s2, fsl], x_[:, s2],
                                         start=(nmm == 0), stop=(nmm == 11),
                                         perf_mode=DR)
                        nmm += 1
                pprec = tst.tile([P, NF], BF16, name="ppr", tag="ppr")
                nc.vector.tensor_scalar(pprec, ps, 1.0 / WSCALE,
                                        bp_s[:, fc:fc + 1], ALU.mult, ALU.add)
                nc.scalar.activation(p8_s[fc // 2][ns][:, fc % 2], ps,
                                     AF.Identity, bias=bp_s[:, fc:fc + 1],
                                     scale=1.0 / WSCALE)
                nc.vector.tensor_sub(pr_s[fc // 2][ns][:, fc % 2], pprec,
                                     p8_s[fc // 2][ns][:, fc % 2])

        # phase-1 operands die here; reuse their space for phase-3 x tiles
        ph1cm.__exit__(None, None, None)
        xnp = es.enter_context(tc.tile_pool(name="xnp", bufs=1))
        xn8_s = xnp.tile([P, NCH, D], E4, name="xn8s", tag="xn8s")
        xnr_s = xnp.tile([P, NCH, D], E4, name="xnrs", tag="xnrs")
        nc.sync.dma_start(out=xn8_s, in_=xn8_d[:])
        nc.sync.dma_start(out=xnr_s, in_=xnr_d[:])

        # att8 (and Ar) per m-chunk-pair: phase-3 stationary APs span two
        # adjacent chunks; writes stream per chunk.
        a8_s = [apool.tile([P, 2, N], E4, name=f"a8_{j}", tag=f"a8_{j}")
                for j in range(NCH // 2)]
        if WEIGHTED_TERMS == 3:
            ar_s = [apool.tile([P, 2, N], E4, name=f"ar_{j}", tag=f"ar_{j}")
                    for j in range(NCH // 2)]

        # -------- Phase 2: scores + softmax --------
        # st[m, n] = T.phi + resid terms (= 2*logit - sum_f phi[m, f])
        # E = exp(0.5*st - 20) bf16 (+ row-sum via accum_out)
        # att8 = e4m3(E * recip)  [ACT Copy with per-partition scale]
        for mc in range(NCH):
            msl = slice(mc * P, (mc + 1) * P)
            sums = stats.tile([P, NSL], F32, name="sums", tag="sums")
            e_t = epool.tile([P, N], BF16, name="et", tag="et")
            for ns in range(NSL):
                nsl = slice(ns * NF, (ns + 1) * NF)
                ps = psum.tile([P, NF], F32, name="pst", tag="pst")
                nmm = 0
                for j in range(FCP):
                    for (sta, mov) in ((p8_s, t8_s), (pr_s, t8_s),
                                       (p8_s, tr_s)):
                        nc.tensor.matmul(
                            ps, sta[j][mc // 4][:, :, (mc % 4) * P:
                                                (mc % 4 + 1) * P],
                            mov[j][ns],
                            start=(nmm == 0), stop=(nmm == 5), perf_mode=DR)
                        nmm += 1
                nc.scalar.activation(e_t[:, nsl], ps, AF.Exp, bias=eb_s,
                                     scale=0.5,
                                     accum_out=sums[:, ns:ns + 1])
            rs = stats.tile([P, 1], F32, name="rs", tag="rs")
            nc.vector.reduce_sum(rs, sums, axis=AX)
            rc = stats.tile([P, 1], F32, name="rc", tag="rc")
            nc.vector.reciprocal(rc, rs)
            # normalization on DVE: ACT is busy with the exp stream, and the
            # per-chunk softmax chain otherwise paces phase 2 above PE rate
            nc.vector.tensor_scalar_mul(a8_s[mc // 2][:, mc % 2], e_t, rc)
            if WEIGHTED_TERMS == 3:
                apre = appool.tile([P, N], BF16, name="ap", tag="ap")
                nc.vector.tensor_scalar_mul(apre, e_t, rc)
                nc.vector.tensor_sub(ar_s[mc // 2][:, mc % 2], apre,
                                     a8_s[mc // 2][:, mc % 2])

        # -------- Phase 3: weighted sum + residual --------
        # out[n, d] = sum_m att[m, n] * x[m, d] + x[n, d]
        for nch in range(NCH):
            nsl128 = slice(nch * P, (nch + 1) * P)
            xrt = xrp.tile([P, D], BF16, name="xrt", tag="xrt")
            nc.sync.dma_start(out=xrt, in_=xr_d[nsl128, :])
            osb = outp.tile([P, D], F32, name="osb", tag="osb")
            nterm = WEIGHTED_TERMS
            last = (nch == NCH - 1)
            # the very last output runs as 512|384|128 pieces: each piece's
            # add+store overlaps the next piece's matmuls, shrinking the
            # post-PE drain to one narrow add + store
            pieces = [NF, NF - P, P] if last else [NF, NF]
            d0 = 0
            for hw_ in pieces:
                dslc = slice(d0, d0 + hw_)
                d0 += hw_
                ps = psum.tile([P, NF], F32, name="pst", tag="pst")
                nmm = 0
                for gp in range(NCH // 2):
                    g2 = slice(2 * gp, 2 * gp + 2)
                    pairs = [(a8_s[gp], xn8_s[:, g2, dslc]),
                             (a8_s[gp], xnr_s[:, g2, dslc])]
                    if nterm == 3:
                        pairs.append((ar_s[gp], xn8_s[:, g2, dslc]))
                    for (sta, mov) in pairs:
                        nc.tensor.matmul(ps[:, 0:hw_], sta[:, :, nsl128], mov,
                                         start=(nmm == 0),
                                         stop=(nmm == 8 * nterm - 1),
                                         perf_mode=DR)
                        nmm += 1
                nc.vector.tensor_add(osb[:, dslc], ps[:, 0:hw_], xrt[:, dslc])
                if last:
                    nc.sync.dma_start(out=out_d[nsl128, dslc],
                                      in_=osb[:, dslc])
            if not last:
                nc.sync.dma_start(out=out_d[nsl128, :], in_=osb)
        es.close()
    nc.finalize()  # Bacc legalization passes (wait splitting, reg alloc, ...)
    return nc


_NC = None


def _get_nc():
    global _NC
    if _NC is None:
        _NC = build_bass()
    return _NC


def _e4(a):
    return np.asarray(a, np.float32).astype(E4NP)


def make_in_maps(x, Wt, bt, Wp, bp):
    def wswz(w):
        # [D, F] -> [P, DCH, F]: [p, dc, f] = w[dc*128+p, f]
        return np.ascontiguousarray(
            w.reshape(DCH, P, F).transpose(1, 0, 2))

    def split_w(W):
        wp = WSCALE * np.asarray(W, np.float32)
        w8 = _e4(wp)
        wr = _e4(wp - w8.astype(np.float32))
        return wswz(w8), wswz(wr)

    wt8, wtr8 = split_w(Wt)
    wp8, wpr8 = split_w(Wp)
    fch = bt.size // P
    bt2 = np.ascontiguousarray(
        (np.asarray(bt, np.float32) / 2).reshape(fch, P).T)
    bp_r = np.ascontiguousarray(np.asarray(bp, np.float32).reshape(fch, P).T)

    common = {"wt8": wt8, "wtr8": wtr8, "wp8": wp8, "wpr8": wpr8,
              "bt2": bt2, "bp": bp_r}

    def xtimg(a):  # [N, D] e4m3 -> per-ns [P, DCH, NF] images of a.T
        at = np.ascontiguousarray(a.T)         # [D, N]
        r = at.reshape(DCH, P, N)
        return [np.ascontiguousarray(r[:, :, ns * NF:(ns + 1) * NF]
                                     .transpose(1, 0, 2))
                for ns in range(NSL)]

    def xnimg(a):  # [N, D] e4m3 -> [P, NCH, D]
        return np.ascontiguousarray(
            a.reshape(NCH, P, D).transpose(1, 0, 2))

    in_maps = []
    for b in range(x.shape[0]):
        xb = np.ascontiguousarray(np.asarray(x[b], np.float32))
        x8 = _e4(xb)
        xr8 = _e4(xb - x8.astype(np.float32))
        m = dict(common)
        for ns, img in enumerate(xtimg(x8)):
            m[f"xt8{ns}"] = img
        for ns, img in enumerate(xtimg(xr8)):
            m[f"xtr8{ns}"] = img
        m["xn8"] = xnimg(x8)
        m["xnr8"] = xnimg(xr8)
        m["xr"] = xb.astype(ml_dtypes.bfloat16)
        in_maps.append(m)
    return in_maps


def run(inputs, trace=False):
    """Run on 8 NeuronCores; returns (out [B,N,D] f32, BassKernelResults)."""
    x = inputs["x"]
    assert x.shape == (B, N, D), x.shape
    nc = _get_nc()
    in_maps = make_in_maps(x, inputs["Wt"], inputs["bt"], inputs["Wp"],
                           inputs["bp"])
    res = run_bass_kernel_spmd(nc, in_maps, core_ids=list(range(B)),
                               trace=trace)
    out = np.stack([res.results[c]["out"] for c in range(B)], axis=0)
    return out.astype(np.float32), res


def kernel(**inputs) -> np.ndarray:
    out, _ = run(inputs)
    return out


# revision 12
# speedup vs baseline: 1.4757x; 1.0073x over previous
"""Trainium2 Bass kernel for nn_AttentionModule (dense_transformer), fp8 DR.

Reference computation (per batch sample b):
    theta = sigmoid(x @ Wt + bt)            # [N, F]
    phi   = x @ Wp + bp                     # [N, F]
    att   = theta @ phi.T                   # [N(n), N(m)]
    att   = softmax(att, axis over n)       # softmax over QUERY axis
    out   = att(n,m) @ x(m,d) + x           # [N, D]

Strategy: pure data parallelism (B=8 samples, one per core, no
collectives) + fp8-e4m3 DoubleRow matmuls (0.5 cycles/row, 2x128
contraction per instruction = 4x bf16 MAC rate in the cost model).

Precision scheme (validated in numerics2.py, rel-l2 vs f64 ref):
 - every bf16 matmul X@W is replaced by 2-3 fp8 terms
       X8@W8 + X8@Wr + Xr@W8      (r = unscaled e4m3 residual)
   accumulated in the same fp32 PSUM group; residual pairs restore
   ~bf16-equivalent precision at 0.5-0.75x of bf16 matmul time.
 - weights are pre-scaled by 32 on host (W' = 32W) so their e4m3
   residuals land in normal range; the 1/32 is folded into the
   activation scale.
 - theta is carried as T = tanh(z/2) = 2*sigmoid(z)-1 in (-1,1):
   the scores logit becomes 0.5*(T.phi) + 0.5*sum_f(phi[m]); the
   second term is constant per m and cancels in the softmax over n
   (exp bias -20; measured exponent range on real inputs [-8, 11.1]).
 - att is quantized to e4m3 AFTER normalization; weighted uses
   2 terms (att8@x8 + att8@xr8, rel~1.38e-2) or 3 terms (+Ar@x8,
   rel~5.5e-3) per WEIGHTED_TERMS.

Scheduling (engine budgets from the TimelineSim cost model):
 - PE (full clock): proj 41us + scores 41us + weighted 55us (2t).
 - PSUM tiles are 2 banks wide; two 512-wide accumulation groups
   share a tile so phase-2 exp runs once per ns-PAIR ([128,1024]
   reads). This halves the per-exp fixed costs (172-cycle PSUM
   access + 187ns accumulator read) that otherwise pace phase 2
   above PE rate (ACT was 799ns/group vs PE 642ns).
 - exp uses accum_out for free row-sums; normalization (att8) runs
   on DVE; residual prep is split ACT/DVE so both stay under PE.
 - tiles are split to consumer granularity (deps are tile-granular):
   W per fc (startup pipelining), T/P per (fc-pair, ns), att8/Ar per
   m-chunk-pair, E per chunk (rotating pool).
 - SBUF: the phase-1 operand pool is closed after phase 1 and its
   space reused for the phase-3 x tiles (LIFO pool stack).
"""

import numpy as np
import ml_dtypes

import concourse.bass as bass
import concourse.bacc as bacc
import concourse.mybir as mybir
from concourse.tile import TileContext
from concourse.bass_utils import run_bass_kernel_spmd

P = 128
B, N, D, F = 8, 2048, 1024, 512
NCH = N // P    # 16 m/n chunks
DCH = D // P    # 8 d chunks
FCH = F // P    # 4 f chunks
NF = 512        # accumulation-group width (half a 2-bank psum tile)
NSL = N // NF   # 4 column slices
DSL = D // NF   # 2 output d slices
WSCALE = 32.0   # host weight pre-scale (residuals out of denormals)

WEIGHTED_TERMS = 2   # 2: att8@(x8+xr8) ~1.4e-2 | 3: +Ar@x8 ~5.5e-3

BF16 = mybir.dt.bfloat16
F32 = mybir.dt.float32
E4 = mybir.dt.float8e4
AX = mybir.AxisListType.X
AF = mybir.ActivationFunctionType
DR = mybir.MatmulPerfMode.DoubleRow
ALU = mybir.AluOpType
E4NP = ml_dtypes.float8_e4m3


def build_bass():
    nc = bacc.Bacc()

    wdefs = [(f"{w}{fc}", w, fc) for w in ("wt8", "wtr8", "wp8", "wpr8")
             for fc in range(FCH)]
    w_d = {name: nc.declare_dram_parameter(name, [P, DCH, P], E4,
                                           isOutput=False)
           for name, _, _ in wdefs}
    bt2_d = nc.declare_dram_parameter("bt2", [P, FCH], F32, isOutput=False)
    bp_d = nc.declare_dram_parameter("bp", [P, FCH], F32, isOutput=False)
    xt8_d = [nc.declare_dram_parameter(f"xt8{ns}", [P, DCH, NF], E4,
                                       isOutput=False) for ns in range(NSL)]
    xtr_d = [nc.declare_dram_parameter(f"xtr8{ns}", [P, DCH, NF], E4,
                                       isOutput=False) for ns in range(NSL)]
    xn8_d = nc.declare_dram_parameter("xn8", [P, NCH, D], E4, isOutput=False)
    xnr_d = nc.declare_dram_parameter("xnr8", [P, NCH, D], E4, isOutput=False)
    xr_d = nc.declare_dram_parameter("xr", [N, D], BF16, isOutput=False)
    out_d = nc.declare_dram_parameter("out", [N, D], F32, isOutput=True)

    with TileContext(nc) as tc:
        from contextlib import ExitStack
        es = ExitStack()
        cpool = es.enter_context(tc.tile_pool(name="const", bufs=1))
        stats = es.enter_context(tc.tile_pool(name="stats", bufs=8))
        epool = es.enter_context(tc.tile_pool(name="ep", bufs=8))
        apool = es.enter_context(tc.tile_pool(name="a8", bufs=1))
        appool = es.enter_context(tc.tile_pool(name="apre", bufs=2))
        xrp = es.enter_context(tc.tile_pool(name="xrp", bufs=3))
        outp = es.enter_context(tc.tile_pool(name="outp", bufs=3))
        tpp = es.enter_context(tc.tile_pool(name="tpp", bufs=1))
        tst = es.enter_context(tc.tile_pool(name="tst", bufs=3))
        # 2-bank psum tiles, two 512-wide groups per tile (zero regions are
        # 2KB, so each half is an independent accumulation group)
        psum = es.enter_context(tc.tile_pool(name="psum", bufs=4,
                                             space="PSUM"))
        ph1cm = tc.tile_pool(name="ph1", bufs=1)
        ph1 = ph1cm.__enter__()

        def ptile():
            return psum.tile([P, 2 * NF], F32, name="pst", tag="pst")

        # ---- constants ----
        bt2_s = cpool.tile([P, FCH], F32, name="bt2s", tag="bt2s")
        bp_s = cpool.tile([P, FCH], F32, name="bps", tag="bps")
        zx = cpool.tile([P, P], BF16, name="zx", tag="zx")
        nc.vector.memset(zx, 0)
        eb_s = cpool.tile([P, 1], F32, name="ebs", tag="ebs")
        nc.vector.memset(eb_s, -20.0)

        # PE warm-up: the pstate ramp holds PE below 2.4GHz for ~3.4us of
        # sustained activity; the first real matmul waits on DMA anyway, so
        # burn the idle time on dummy matmuls (costless: PE was idle).
        NWARM = 88  # 128-wide dummies: cover ~6.5us of startup DMA latency
        zp = ptile()
        for i in range(NWARM):
            nc.tensor.matmul(zp[:, 0:P], zx, zx, start=(i == 0),
                             stop=(i == NWARM - 1))

        # ---- phase-1 operand tiles + DMAs (first-use order) ----
        w_s = {name: ph1.tile([P, DCH, P], E4, name=name, tag=name)
               for name, _, _ in wdefs}
        xt8_s = [ph1.tile([P, DCH, NF], E4, name=f"xt8{ns}", tag=f"xt8{ns}")
                 for ns in range(NSL)]
        xtr_s = [ph1.tile([P, DCH, NF], E4, name=f"xtr{ns}", tag=f"xtr{ns}")
                 for ns in range(NSL)]

        def ld(name):
            nc.sync.dma_start(out=w_s[name], in_=w_d[name][:])

        ld("wt80")
        ld("wtr80")
        nc.sync.dma_start(out=xt8_s[0], in_=xt8_d[0][:])
        nc.sync.dma_start(out=xtr_s[0], in_=xtr_d[0][:])
        nc.sync.dma_start(out=bt2_s, in_=bt2_d[:])
        for fc in range(1, FCH):
            ld(f"wt8{fc}")
            ld(f"wtr8{fc}")
        for fc in range(FCH):
            ld(f"wp8{fc}")
            ld(f"wpr8{fc}")
        nc.sync.dma_start(out=bp_s, in_=bp_d[:])
        for ns in range(1, NSL):
            nc.sync.dma_start(out=xt8_s[ns], in_=xt8_d[ns][:])
            nc.sync.dma_start(out=xtr_s[ns], in_=xtr_d[ns][:])

        # T/P operand tiles: per (fc-pair, ns-512) so scores(mc, ns) only
        # waits on the exact phase-1 blocks it reads.
        FCP = FCH // 2
        t8_s = [[tpp.tile([P, 2, NF], E4, name=f"t8_{j}_{ns}",
                          tag=f"t8_{j}_{ns}") for ns in range(NSL)]
                for j in range(FCP)]
        tr_s = [[tpp.tile([P, 2, NF], E4, name=f"tr_{j}_{ns}",
                          tag=f"tr_{j}_{ns}") for ns in range(NSL)]
                for j in range(FCP)]
        p8_s = [[tpp.tile([P, 2, NF], E4, name=f"p8_{j}_{ns}",
                          tag=f"p8_{j}_{ns}") for ns in range(NSL)]
                for j in range(FCP)]
        pr_s = [[tpp.tile([P, 2, NF], E4, name=f"pr_{j}_{ns}",
                          tag=f"pr_{j}_{ns}") for ns in range(NSL)]
                for j in range(FCP)]

        # -------- Phase 1: projections (3-term fp8 DR) --------
        # psum = x8@W8' + x8@Wr' + xr8@W8'   (W' = 32W; 12 DR per group)
        # T = tanh(psum/64 + bt/2)  -> bf16 staging + e4m3 + residual
        # phi = psum/32 + bp        -> same
        pt = None
        for ns in range(NSL):
            for gi, (proj, fc) in enumerate(
                    [("t", fc) for fc in range(FCH)] +
                    [("p", fc) for fc in range(FCH)]):
                if gi % 2 == 0:
                    pt = ptile()
                ps = pt[:, (gi % 2) * NF:(gi % 2 + 1) * NF]
                wm, wr = (("wt8", "wtr8") if proj == "t" else ("wp8", "wpr8"))
                nmm = 0
                for dcp in range(DCH // 2):
                    s2 = slice(2 * dcp, 2 * dcp + 2)
                    for (w_, x_) in ((wm, xt8_s[ns]), (wr, xt8_s[ns]),
                                     (wm, xtr_s[ns])):
                        nc.tensor.matmul(ps, w_s[f"{w_}{fc}"][:, s2],
                                         x_[:, s2],
                                         start=(nmm == 0), stop=(nmm == 11),
                                         perf_mode=DR)
                        nmm += 1
                j, h = fc // 2, fc % 2
                if proj == "t":
                    tprec = tst.tile([P, NF], BF16, name="tpr", tag="tpr")
                    nc.scalar.activation(tprec, ps, AF.Tanh,
                                         bias=bt2_s[:, fc:fc + 1],
                                         scale=1.0 / (2 * WSCALE))
                    nc.scalar.activation(t8_s[j][ns][:, h], ps, AF.Tanh,
                                         bias=bt2_s[:, fc:fc + 1],
                                         scale=1.0 / (2 * WSCALE))
                    nc.vector.tensor_sub(tr_s[j][ns][:, h], tprec,
                                         t8_s[j][ns][:, h])
                else:
                    pprec = tst.tile([P, NF], BF16, name="ppr", tag="ppr")
                    nc.vector.tensor_scalar(pprec, ps, 1.0 / WSCALE,
                                            bp_s[:, fc:fc + 1], ALU.mult,
                                            ALU.add)
                    nc.scalar.activation(p8_s[j][ns][:, h], ps, AF.Identity,
                                         bias=bp_s[:, fc:fc + 1],
                                         scale=1.0 / WSCALE)
                    nc.vector.tensor_sub(pr_s[j][ns][:, h], pprec,
                                         p8_s[j][ns][:, h])

        # phase-1 operands die here; reuse their space for phase-3 x tiles
        ph1cm.__exit__(None, None, None)
        xnp = es.enter_context(tc.tile_pool(name="xnp", bufs=1))
        xn8_s = xnp.tile([P, NCH, D], E4, name="xn8s", tag="xn8s")
        xnr_s = xnp.tile([P, NCH, D], E4, name="xnrs", tag="xnrs")
        nc.sync.dma_start(out=xn8_s, in_=xn8_d[:])
        nc.sync.dma_start(out=xnr_s, in_=xnr_d[:])

        # att8 (and Ar) per m-chunk-pair: phase-3 stationary APs span two
        # adjacent chunks; writes stream per chunk.
        a8_s = [apool.tile([P, 2, N], E4, name=f"a8_{j}", tag=f"a8_{j}")
                for j in range(NCH // 2)]
        if WEIGHTED_TERMS == 3:
            ar_s = [apool.tile([P, 2, N], E4, name=f"ar_{j}", tag=f"ar_{j}")
                    for j in range(NCH // 2)]

        # -------- Phase 2: scores + softmax --------
        # st[m, n] = T.phi + resid terms (= 2*logit - sum_f phi[m, f])
        # E = exp(0.5*st - 20) bf16, one op per ns-PAIR ([128,1024] from a
        # full 2-bank tile), row-sums via accum_out
        # att8 = e4m3(E * recip) on DVE
        for mc in range(NCH):
            sums = stats.tile([P, 2], F32, name="sums", tag="sums")
            e_t = epool.tile([P, N], BF16, name="et", tag="et")
            for nsp in range(NSL // 2):
                pt = ptile()
                for nsh in range(2):
                    ns = 2 * nsp + nsh
                    ps = pt[:, nsh * NF:(nsh + 1) * NF]
                    nmm = 0
                    for j in range(FCP):
                        for (sta, mov) in ((p8_s, t8_s), (pr_s, t8_s),
                                           (p8_s, tr_s)):
                            nc.tensor.matmul(
                                ps, sta[j][mc // 4][:, :, (mc % 4) * P:
                                                    (mc % 4 + 1) * P],
                                mov[j][ns],
                                start=(nmm == 0), stop=(nmm == 5),
                                perf_mode=DR)
                            nmm += 1
                nc.scalar.activation(
                    e_t[:, nsp * 2 * NF:(nsp + 1) * 2 * NF], pt, AF.Exp,
                    bias=eb_s, scale=0.5,
                    accum_out=sums[:, nsp:nsp + 1])
            rs = stats.tile([P, 1], F32, name="rs", tag="rs")
            nc.vector.reduce_sum(rs, sums, axis=AX)
            rc = stats.tile([P, 1], F32, name="rc", tag="rc")
            nc.vector.reciprocal(rc, rs)
            # normalization on DVE: ACT is saturated by the exp stream
            nc.vector.tensor_scalar_mul(a8_s[mc // 2][:, mc % 2], e_t, rc)
            if WEIGHTED_TERMS == 3:
                apre = appool.tile([P, N], BF16, name="ap", tag="ap")
                nc.vector.tensor_scalar_mul(apre, e_t, rc)
                nc.vector.tensor_sub(ar_s[mc // 2][:, mc % 2], apre,
                                     a8_s[mc // 2][:, mc % 2])

        # -------- Phase 3: weighted sum + residual --------
        # out[n, d] = sum_m att[m, n] * x[m, d] + x[n, d]
        nterm = WEIGHTED_TERMS
        for nch in range(NCH):
            nsl128 = slice(nch * P, (nch + 1) * P)
            xrt = xrp.tile([P, D], BF16, name="xrt", tag="xrt")
            nc.sync.dma_start(out=xrt, in_=xr_d[nsl128, :])
            osb = outp.tile([P, D], F32, name="osb", tag="osb")
            last = (nch == NCH - 1)
            # the very last output runs as 512|384|128 pieces: each piece's
            # add+store overlaps the next piece's matmuls, shrinking the
            # post-PE drain to one narrow add + store
            pieces = [NF, NF - P, P] if last else [NF, NF]
            d0 = 0
            pt = None
            for pi, hw_ in enumerate(pieces):
                dslc = slice(d0, d0 + hw_)
                d0 += hw_
                if pi % 2 == 0:
                    pt = ptile()
                ps = pt[:, (pi % 2) * NF:(pi % 2) * NF + hw_]
                nmm = 0
                for gp in range(NCH // 2):
                    g2 = slice(2 * gp, 2 * gp + 2)
                    pairs = [(a8_s[gp], xn8_s[:, g2, dslc]),
                             (a8_s[gp], xnr_s[:, g2, dslc])]
                    if nterm == 3:
                        pairs.append((ar_s[gp], xn8_s[:, g2, dslc]))
                    for (sta, mov) in pairs:
                        nc.tensor.matmul(ps, sta[:, :, nsl128], mov,
                                         start=(nmm == 0),
                                         stop=(nmm == 8 * nterm - 1),
                                         perf_mode=DR)
                        nmm += 1
                if last:
                    nc.vector.tensor_add(osb[:, dslc], ps, xrt[:, dslc])
                    nc.sync.dma_start(out=out_d[nsl128, dslc],
                                      in_=osb[:, dslc])
            if not last:
                nc.vector.tensor_add(osb, pt, xrt)
                nc.sync.dma_start(out=out_d[nsl128, :], in_=osb)
        es.close()
    nc.finalize()  # Bacc legalization passes (wait splitting, reg alloc, ...)
    return nc


_NC = None


def _get_nc():
    global _NC
    if _NC is None:
        _NC = build_bass()
    return _NC


def _e4(a):
    return np.asarray(a, np.float32).astype(E4NP)


def make_in_maps(x, Wt, bt, Wp, bp):
    def wswz(w, fc):
        # [D, F] -> per-fc [P, DCH, P]: [p, dc, fw] = w[dc*128+p, fc*128+fw]
        blk = w[:, fc * P:(fc + 1) * P]
        return np.ascontiguousarray(blk.reshape(DCH, P, P).transpose(1, 0, 2))

    def split_w(W):
        wp = WSCALE * np.asarray(W, np.float32)
        w8 = _e4(wp)
        wr = _e4(wp - w8.astype(np.float32))
        return w8, wr

    wt8, wtr8 = split_w(Wt)
    wp8, wpr8 = split_w(Wp)
    fch = bt.size // P
    bt2 = np.ascontiguousarray(
        (np.asarray(bt, np.float32) / 2).reshape(fch, P).T)
    bp_r = np.ascontiguousarray(np.asarray(bp, np.float32).reshape(fch, P).T)

    common = {"bt2": bt2, "bp": bp_r}
    for nm, arr in (("wt8", wt8), ("wtr8", wtr8), ("wp8", wp8),
                    ("wpr8", wpr8)):
        for fc in range(FCH):
            common[f"{nm}{fc}"] = wswz(arr, fc)

    def xtimg(a):  # [N, D] e4m3 -> per-ns [P, DCH, NF] images of a.T
        at = np.ascontiguousarray(a.T)         # [D, N]
        r = at.reshape(DCH, P, N)
        return [np.ascontiguousarray(r[:, :, ns * NF:(ns + 1) * NF]
                                     .transpose(1, 0, 2))
                for ns in range(NSL)]

    def xnimg(a):  # [N, D] e4m3 -> [P, NCH, D]
        return np.ascontiguousarray(
            a.reshape(NCH, P, D).transpose(1, 0, 2))

    in_maps = []
    for b in range(x.shape[0]):
        xb = np.ascontiguousarray(np.asarray(x[b], np.float32))
        x8 = _e4(xb)
        xr8 = _e4(xb - x8.astype(np.float32))
        m = dict(common)
        for ns, img in enumerate(xtimg(x8)):
            m[f"xt8{ns}"] = img
        for ns, img in enumerate(xtimg(xr8)):
            m[f"xtr8{ns}"] = img
        m["xn8"] = xnimg(x8)
        m["xnr8"] = xnimg(xr8)
        m["xr"] = xb.astype(ml_dtypes.bfloat16)
        in_maps.append(m)
    return in_maps


def run(inputs, trace=False):
    """Run on 8 NeuronCores; returns (out [B,N,D] f32, BassKernelResults)."""
    x = inputs["x"]
    assert x.shape == (B, N, D), x.shape
    nc = _get_nc()
    in_maps = make_in_maps(x, inputs["Wt"], inputs["bt"], inputs["Wp"],
                           inputs["bp"])
    res = run_bass_kernel_spmd(nc, in_maps, core_ids=list(range(B)),
                               trace=trace)
    out = np.stack([res.results[c]["out"] for c in range(B)], axis=0)
    return out.astype(np.float32), res


def kernel(**inputs) -> np.ndarray:
    out, _ = run(inputs)
    return out


# revision 14
# speedup vs baseline: 1.4876x; 1.0080x over previous
"""Trainium2 Bass kernel for nn_AttentionModule (dense_transformer), fp8 DR.

Reference computation (per batch sample b):
    theta = sigmoid(x @ Wt + bt)            # [N, F]
    phi   = x @ Wp + bp                     # [N, F]
    att   = theta @ phi.T                   # [N(n), N(m)]
    att   = softmax(att, axis over n)       # softmax over QUERY axis
    out   = att(n,m) @ x(m,d) + x           # [N, D]

Strategy: pure data parallelism (B=8 samples, one per core, no
collectives) + fp8-e4m3 DoubleRow matmuls (0.5 cycles/row, 2x128
contraction per instruction = 4x bf16 MAC rate in the cost model).

Precision scheme (validated in numerics2.py, rel-l2 vs f64 ref):
 - every bf16 matmul X@W is replaced by 2-3 fp8 terms
       X8@W8 + X8@Wr + Xr@W8      (r = unscaled e4m3 residual)
   accumulated in the same fp32 PSUM group; residual pairs restore
   ~bf16-equivalent precision at 0.5-0.75x of bf16 matmul time.
 - weights are pre-scaled by 32 on host (W' = 32W) so their e4m3
   residuals land in normal range; the 1/32 is folded into the
   activation scale.
 - theta is carried as T = tanh(z/2) = 2*sigmoid(z)-1 in (-1,1):
   the scores logit becomes 0.5*(T.phi) + 0.5*sum_f(phi[m]); the
   second term is constant per m and cancels in the softmax over n
   (exp bias -20; measured exponent range on real inputs [-8, 11.1]).
 - att is quantized to e4m3 AFTER normalization; weighted uses
   2 terms (att8@x8 + att8@xr8, rel~1.38e-2) or 3 terms (+Ar@x8,
   rel~5.5e-3) per WEIGHTED_TERMS.

Scheduling (engine budgets from the TimelineSim cost model):
 - PE (full clock): proj 41us + scores 41us + weighted 55us (2t).
 - PSUM tiles are 2 banks wide; two 512-wide accumulation groups
   share a tile so phase-2 exp runs once per ns-PAIR ([128,1024]
   reads). This halves the per-exp fixed costs (172-cycle PSUM
   access + 187ns accumulator read) that otherwise pace phase 2
   above PE rate (ACT was 799ns/group vs PE 642ns).
 - exp uses accum_out for free row-sums; normalization (att8) runs
   on DVE; residual prep is split ACT/DVE so both stay under PE.
 - tiles are split to consumer granularity (deps are tile-granular):
   W per fc (startup pipelining), T/P per (fc-pair, ns), att8/Ar per
   m-chunk-pair, E per chunk (rotating pool).
 - SBUF: the phase-1 operand pool is closed after phase 1 and its
   space reused for the phase-3 x tiles (LIFO pool stack).
"""

import numpy as np
import ml_dtypes

import concourse.bass as bass
import concourse.bacc as bacc
import concourse.mybir as mybir
from concourse.tile import TileContext
from concourse.bass_utils import run_bass_kernel_spmd

P = 128
B, N, D, F = 8, 2048, 1024, 512
NCH = N // P    # 16 m/n chunks
DCH = D // P    # 8 d chunks
FCH = F // P    # 4 f chunks
NF = 512        # accumulation-group width (half a 2-bank psum tile)
NSL = N // NF   # 4 column slices
DSL = D // NF   # 2 output d slices
WSCALE = 32.0   # host weight pre-scale (residuals out of denormals)

WEIGHTED_TERMS = 2   # 2: att8@(x8+xr8) ~1.4e-2 | 3: +Ar@x8 ~5.5e-3

BF16 = mybir.dt.bfloat16
F32 = mybir.dt.float32
E4 = mybir.dt.float8e4
AX = mybir.AxisListType.X
AF = mybir.ActivationFunctionType
DR = mybir.MatmulPerfMode.DoubleRow
ALU = mybir.AluOpType
E4NP = ml_dtypes.float8_e4m3


def build_bass():
    nc = bacc.Bacc()

    wdefs = [(f"{w}{fc}", w, fc) for w in ("wt8", "wtr8", "wp8", "wpr8")
             for fc in range(FCH)]
    w_d = {name: nc.declare_dram_parameter(name, [P, DCH, P], E4,
                                           isOutput=False)
           for name, _, _ in wdefs}
    bt2_d = nc.declare_dram_parameter("bt2", [P, FCH], F32, isOutput=False)
    bp_d = nc.declare_dram_parameter("bp", [P, FCH], F32, isOutput=False)
    xt8_d = [nc.declare_dram_parameter(f"xt8{ns}", [P, DCH, NF], E4,
                                       isOutput=False) for ns in range(NSL)]
    xtr_d = [nc.declare_dram_parameter(f"xtr8{ns}", [P, DCH, NF], E4,
                                       isOutput=False) for ns in range(NSL)]
    xn8_d = nc.declare_dram_parameter("xn8", [P, NCH, D], E4, isOutput=False)
    xnr_d = nc.declare_dram_parameter("xnr8", [P, NCH, D], E4, isOutput=False)
    xr_d = nc.declare_dram_parameter("xr", [N, D], BF16, isOutput=False)
    out_d = nc.declare_dram_parameter("out", [N, D], F32, isOutput=True)

    with TileContext(nc) as tc:
        from contextlib import ExitStack
        es = ExitStack()
        cpool = es.enter_context(tc.tile_pool(name="const", bufs=1))
        stats = es.enter_context(tc.tile_pool(name="stats", bufs=8))
        epool = es.enter_context(tc.tile_pool(name="ep", bufs=8))
        apool = es.enter_context(tc.tile_pool(name="a8", bufs=1))
        appool = es.enter_context(tc.tile_pool(name="apre", bufs=2))
        xrp = es.enter_context(tc.tile_pool(name="xrp", bufs=3))
        outp = es.enter_context(tc.tile_pool(name="outp", bufs=3))
        tpp = es.enter_context(tc.tile_pool(name="tpp", bufs=1))
        tst = es.enter_context(tc.tile_pool(name="tst", bufs=4))
        # 2-bank psum tiles, two 512-wide groups per tile (zero regions are
        # 2KB, so each half is an independent accumulation group)
        psum = es.enter_context(tc.tile_pool(name="psum", bufs=4,
                                             space="PSUM"))
        ph1cm = tc.tile_pool(name="ph1", bufs=1)
        ph1 = ph1cm.__enter__()

        def ptile():
            return psum.tile([P, 2 * NF], F32, name="pst", tag="pst")

        # ---- constants ----
        bt2_s = cpool.tile([P, FCH], F32, name="bt2s", tag="bt2s")
        bp_s = cpool.tile([P, FCH], F32, name="bps", tag="bps")
        zx = cpool.tile([P, P], BF16, name="zx", tag="zx")
        nc.vector.memset(zx, 0)
        eb_s = cpool.tile([P, 1], F32, name="ebs", tag="ebs")
        nc.vector.memset(eb_s, -20.0)

        # PE warm-up: the pstate ramp holds PE below 2.4GHz for ~3.4us of
        # sustained activity; the first real matmul waits on DMA anyway, so
        # burn the idle time on dummy matmuls (costless: PE was idle).
        NWARM = 88  # 128-wide dummies: cover ~6.5us of startup DMA latency
        zp = ptile()
        for i in range(NWARM):
            nc.tensor.matmul(zp[:, 0:P], zx, zx, start=(i == 0),
                             stop=(i == NWARM - 1))

        # ---- phase-1 operand tiles + DMAs (first-use order) ----
        w_s = {name: ph1.tile([P, DCH, P], E4, name=name, tag=name)
               for name, _, _ in wdefs}
        xt8_s = [ph1.tile([P, DCH, NF], E4, name=f"xt8{ns}", tag=f"xt8{ns}")
                 for ns in range(NSL)]
        xtr_s = [ph1.tile([P, DCH, NF], E4, name=f"xtr{ns}", tag=f"xtr{ns}")
                 for ns in range(NSL)]

        def ld(name):
            nc.sync.dma_start(out=w_s[name], in_=w_d[name][:])

        ld("wt80")
        ld("wtr80")
        nc.sync.dma_start(out=xt8_s[0], in_=xt8_d[0][:])
        nc.sync.dma_start(out=xtr_s[0], in_=xtr_d[0][:])
        nc.sync.dma_start(out=bt2_s, in_=bt2_d[:])
        for fc in range(1, FCH):
            ld(f"wt8{fc}")
            ld(f"wtr8{fc}")
        for fc in range(FCH):
            ld(f"wp8{fc}")
            ld(f"wpr8{fc}")
        nc.sync.dma_start(out=bp_s, in_=bp_d[:])
        for ns in range(1, NSL):
            nc.sync.dma_start(out=xt8_s[ns], in_=xt8_d[ns][:])
            nc.sync.dma_start(out=xtr_s[ns], in_=xtr_d[ns][:])

        # T/P operand tiles: per (fc-pair, ns-512) so scores(mc, ns) only
        # waits on the exact phase-1 blocks it reads.
        FCP = FCH // 2
        t8_s = [[tpp.tile([P, 2, NF], E4, name=f"t8_{j}_{ns}",
                          tag=f"t8_{j}_{ns}") for ns in range(NSL)]
                for j in range(FCP)]
        tr_s = [[tpp.tile([P, 2, NF], E4, name=f"tr_{j}_{ns}",
                          tag=f"tr_{j}_{ns}") for ns in range(NSL)]
                for j in range(FCP)]
        p8_s = [[tpp.tile([P, 2, NF], E4, name=f"p8_{j}_{ns}",
                          tag=f"p8_{j}_{ns}") for ns in range(NSL)]
                for j in range(FCP)]
        pr_s = [[tpp.tile([P, 2, NF], E4, name=f"pr_{j}_{ns}",
                          tag=f"pr_{j}_{ns}") for ns in range(NSL)]
                for j in range(FCP)]

        # -------- Phase 1: projections (3-term fp8 DR) --------
        # psum = x8@W8' + x8@Wr' + xr8@W8'   (W' = 32W; 12 DR per group)
        # T = tanh(psum/64 + bt/2)  -> bf16 staging + e4m3 + residual
        # phi = psum/32 + bp        -> same
        pt = None
        for ns in range(NSL):
            # theta/phi interleaved: ACT sees tanh,tanh,id,... instead of an
            # 8-tanh bunch followed by 4 ids — smoother consumer flow for the
            # psum-tile rotation
            for gi, (proj, fc) in enumerate(
                    [(pr_, fc) for fc in range(FCH) for pr_ in ("t", "p")]):
                if gi % 2 == 0:
                    pt = ptile()
                ps = pt[:, (gi % 2) * NF:(gi % 2 + 1) * NF]
                wm, wr = (("wt8", "wtr8") if proj == "t" else ("wp8", "wpr8"))
                nmm = 0
                for dcp in range(DCH // 2):
                    s2 = slice(2 * dcp, 2 * dcp + 2)
                    for (w_, x_) in ((wm, xt8_s[ns]), (wr, xt8_s[ns]),
                                     (wm, xtr_s[ns])):
                        nc.tensor.matmul(ps, w_s[f"{w_}{fc}"][:, s2],
                                         x_[:, s2],
                                         start=(nmm == 0), stop=(nmm == 11),
                                         perf_mode=DR)
                        nmm += 1
                j, h = fc // 2, fc % 2
                if proj == "t":
                    tprec = tst.tile([P, NF], BF16, name="tpr", tag="tpr")
                    nc.scalar.activation(tprec, ps, AF.Tanh,
                                         bias=bt2_s[:, fc:fc + 1],
                                         scale=1.0 / (2 * WSCALE))
                    nc.scalar.activation(t8_s[j][ns][:, h], ps, AF.Tanh,
                                         bias=bt2_s[:, fc:fc + 1],
                                         scale=1.0 / (2 * WSCALE))
                    nc.vector.tensor_sub(tr_s[j][ns][:, h], tprec,
                                         t8_s[j][ns][:, h])
                else:
                    pprec = tst.tile([P, NF], BF16, name="ppr", tag="ppr")
                    nc.vector.tensor_scalar(pprec, ps, 1.0 / WSCALE,
                                            bp_s[:, fc:fc + 1], ALU.mult,
                                            ALU.add)
                    nc.scalar.activation(p8_s[j][ns][:, h], ps, AF.Identity,
                                         bias=bp_s[:, fc:fc + 1],
                                         scale=1.0 / WSCALE)
                    nc.vector.tensor_sub(pr_s[j][ns][:, h], pprec,
                                         p8_s[j][ns][:, h])

        # phase-1 operands die here; reuse their space for phase-3 x tiles
        ph1cm.__exit__(None, None, None)
        xnp = es.enter_context(tc.tile_pool(name="xnp", bufs=1))
        xn8_s = xnp.tile([P, NCH, D], E4, name="xn8s", tag="xn8s")
        xnr_s = xnp.tile([P, NCH, D], E4, name="xnrs", tag="xnrs")
        nc.sync.dma_start(out=xn8_s, in_=xn8_d[:])
        nc.sync.dma_start(out=xnr_s, in_=xnr_d[:])

        # att8 (and Ar) per m-chunk-pair: phase-3 stationary APs span two
        # adjacent chunks; writes stream per chunk.
        a8_s = [apool.tile([P, 2, N], E4, name=f"a8_{j}", tag=f"a8_{j}")
                for j in range(NCH // 2)]
        if WEIGHTED_TERMS == 3:
            ar_s = [apool.tile([P, 2, N], E4, name=f"ar_{j}", tag=f"ar_{j}")
                    for j in range(NCH // 2)]

        # -------- Phase 2: scores + softmax --------
        # st[m, n] = T.phi + resid terms (= 2*logit - sum_f phi[m, f])
        # E = exp(0.5*st - 20) bf16, one op per ns-PAIR ([128,1024] from a
        # full 2-bank tile), row-sums via accum_out
        # att8 = e4m3(E * recip) on DVE
        for mc in range(NCH):
            sums = stats.tile([P, 2], F32, name="sums", tag="sums")
            e_t = epool.tile([P, N], BF16, name="et", tag="et")
            for nsp in range(NSL // 2):
                pt = ptile()
                for nsh in range(2):
                    ns = 2 * nsp + nsh
                    ps = pt[:, nsh * NF:(nsh + 1) * NF]
                    nmm = 0
                    for j in range(FCP):
                        for (sta, mov) in ((p8_s, t8_s), (pr_s, t8_s),
                                           (p8_s, tr_s)):
                            nc.tensor.matmul(
                                ps, sta[j][mc // 4][:, :, (mc % 4) * P:
                                                    (mc % 4 + 1) * P],
                                mov[j][ns],
                                start=(nmm == 0), stop=(nmm == 5),
                                perf_mode=DR)
                            nmm += 1
                nc.scalar.activation(
                    e_t[:, nsp * 2 * NF:(nsp + 1) * 2 * NF], pt, AF.Exp,
                    bias=eb_s, scale=0.5,
                    accum_out=sums[:, nsp:nsp + 1])
            rs = stats.tile([P, 1], F32, name="rs", tag="rs")
            nc.vector.reduce_sum(rs, sums, axis=AX)
            rc = stats.tile([P, 1], F32, name="rc", tag="rc")
            nc.vector.reciprocal(rc, rs)
            # normalization on DVE: ACT is saturated by the exp stream
            nc.vector.tensor_scalar_mul(a8_s[mc // 2][:, mc % 2], e_t, rc)
            if WEIGHTED_TERMS == 3:
                apre = appool.tile([P, N], BF16, name="ap", tag="ap")
                nc.vector.tensor_scalar_mul(apre, e_t, rc)
                nc.vector.tensor_sub(ar_s[mc // 2][:, mc % 2], apre,
                                     a8_s[mc // 2][:, mc % 2])

        # -------- Phase 3: weighted sum + residual --------
        # out[n, d] = sum_m att[m, n] * x[m, d] + x[n, d]
        nterm = WEIGHTED_TERMS
        for nch in range(NCH):
            nsl128 = slice(nch * P, (nch + 1) * P)
            xrt = xrp.tile([P, D], BF16, name="xrt", tag="xrt")
            nc.sync.dma_start(out=xrt, in_=xr_d[nsl128, :])
            osb = outp.tile([P, D], F32, name="osb", tag="osb")
            last = (nch == NCH - 1)
            # the very last output runs as 512|384|128 pieces: each piece's
            # add+store overlaps the next piece's matmuls, shrinking the
            # post-PE drain to one narrow add + store
            pieces = [NF, NF - P, P] if last else [NF, NF]
            d0 = 0
            pt = None
            for pi, hw_ in enumerate(pieces):
                dslc = slice(d0, d0 + hw_)
                d0 += hw_
                if pi % 2 == 0:
                    pt = ptile()
                ps = pt[:, (pi % 2) * NF:(pi % 2) * NF + hw_]
                nmm = 0
                for gp in range(NCH // 2):
                    g2 = slice(2 * gp, 2 * gp + 2)
                    pairs = [(a8_s[gp], xn8_s[:, g2, dslc]),
                             (a8_s[gp], xnr_s[:, g2, dslc])]
                    if nterm == 3:
                        pairs.append((ar_s[gp], xn8_s[:, g2, dslc]))
                    for (sta, mov) in pairs:
                        nc.tensor.matmul(ps, sta[:, :, nsl128], mov,
                                         start=(nmm == 0),
                                         stop=(nmm == 8 * nterm - 1),
                                         perf_mode=DR)
                        nmm += 1
                if last:
                    nc.vector.tensor_add(osb[:, dslc], ps, xrt[:, dslc])
                    nc.sync.dma_start(out=out_d[nsl128, dslc],
                                      in_=osb[:, dslc])
            if not last:
                nc.vector.tensor_add(osb, pt, xrt)
                nc.sync.dma_start(out=out_d[nsl128, :], in_=osb)
        es.close()
    nc.finalize()  # Bacc legalization passes (wait splitting, reg alloc, ...)
    return nc


_NC = None


def _get_nc():
    global _NC
    if _NC is None:
        _NC = build_bass()
    return _NC


def _e4(a):
    return np.asarray(a, np.float32).astype(E4NP)


def make_in_maps(x, Wt, bt, Wp, bp):
    def wswz(w, fc):
        # [D, F] -> per-fc [P, DCH, P]: [p, dc, fw] = w[dc*128+p, fc*128+fw]
        blk = w[:, fc * P:(fc + 1) * P]
        return np.ascontiguousarray(blk.reshape(DCH, P, P).transpose(1, 0, 2))

    def split_w(W):
        wp = WSCALE * np.asarray(W, np.float32)
        w8 = _e4(wp)
        wr = _e4(wp - w8.astype(np.float32))
        return w8, wr

    wt8, wtr8 = split_w(Wt)
    wp8, wpr8 = split_w(Wp)
    fch = bt.size // P
    bt2 = np.ascontiguousarray(
        (np.asarray(bt, np.float32) / 2).reshape(fch, P).T)
    bp_r = np.ascontiguousarray(np.asarray(bp, np.float32).reshape(fch, P).T)

    common = {"bt2": bt2, "bp": bp_r}
    for nm, arr in (("wt8", wt8), ("wtr8", wtr8), ("wp8", wp8),
                    ("wpr8", wpr8)):
        for fc in range(FCH):
            common[f"{nm}{fc}"] = wswz(arr, fc)

    def xtimg(a):  # [N, D] e4m3 -> per-ns [P, DCH, NF] images of a.T
        at = np.ascontiguousarray(a.T)         # [D, N]
        r = at.reshape(DCH, P, N)
        return [np.ascontiguousarray(r[:, :, ns * NF:(ns + 1) * NF]
                                     .transpose(1, 0, 2))
                for ns in range(NSL)]

    def xnimg(a):  # [N, D] e4m3 -> [P, NCH, D]
        return np.ascontiguousarray(
            a.reshape(NCH, P, D).transpose(1, 0, 2))

    in_maps = []
    for b in range(x.shape[0]):
        xb = np.ascontiguousarray(np.asarray(x[b], np.float32))
        x8 = _e4(xb)
        xr8 = _e4(xb - x8.astype(np.float32))
        m = dict(common)
        for ns, img in enumerate(xtimg(x8)):
            m[f"xt8{ns}"] = img
        for ns, img in enumerate(xtimg(xr8)):
            m[f"xtr8{ns}"] = img
        m["xn8"] = xnimg(x8)
        m["xnr8"] = xnimg(xr8)
        m["xr"] = xb.astype(ml_dtypes.bfloat16)
        in_maps.append(m)
    return in_maps


def run(inputs, trace=False):
    """Run on 8 NeuronCores; returns (out [B,N,D] f32, BassKernelResults)."""
    x = inputs["x"]
    assert x.shape == (B, N, D), x.shape
    nc = _get_nc()
    in_maps = make_in_maps(x, inputs["Wt"], inputs["bt"], inputs["Wp"],
                           inputs["bp"])
    res = run_bass_kernel_spmd(nc, in_maps, core_ids=list(range(B)),
                               trace=trace)
    out = np.stack([res.results[c]["out"] for c in range(B)], axis=0)
    return out.astype(np.float32), res


def kernel(**inputs) -> np.ndarray:
    out, _ = run(inputs)
    return out
